# revision 1
# baseline (speedup 1.0000x reference)
"""AccessibilitySVIGNN Trainium2 kernel (8-core SPMD).

See bottom of file for entry point `kernel(**inputs)`.

Design:
- Nodes dst-sharded across 8 cores; per-core slot grid (nodes degree-sorted
  into 128-partition groups, uniform per-group column count across cores so
  one SPMD program works for all cores). Every edge incl. self-loop = 1 slot.
- Gather tables pack 4 nodes per row so an int16 pack index covers 100k nodes;
  a host-built one-hot mask selects the quadrant on the vector engine.
- GCN normalization via pre-scaled tables (1/sqrt(deg_src)) + post-scale.
- GAT via gathered [h | alpha_src] rows + online plain-exp softmax.
- Dense per-node compute feature-transposed on the tensor engine; conv tables
  exchanged with AllGather collectives.
"""

import math
import os

import numpy as np

EPS = 1e-5
CORES = 8
WIN = 8  # gather window: 8 cols x 128 partitions = 1024 idxs per dma_gather


class Plan:
    pass


def build_plan(edge_index, n_nodes):
    p = Plan()
    N = n_nodes
    src = edge_index[0].astype(np.int64)
    dst = edge_index[1].astype(np.int64)
    SH = N // CORES
    NG = math.ceil(SH / 128)
    NSLOT = NG * 128
    p.N, p.SH, p.NG, p.NSLOT = N, SH, NG, NSLOT

    counts = np.bincount(dst, minlength=N) + 1  # incl self-loop

    orders = np.full((CORES, NSLOT), -1, np.int64)
    Ks = np.zeros((CORES, NG), np.int64)
    for c in range(CORES):
        nodes = np.arange(c * SH, (c + 1) * SH)
        o = np.argsort(-counts[nodes], kind="stable")
        orders[c, :SH] = nodes[o]
        cnt_sorted = np.zeros(NSLOT, np.int64)
        cnt_sorted[:SH] = counts[nodes][o]
        Ks[c] = cnt_sorted.reshape(NG, 128).max(1)
    Kg = np.maximum(Ks.max(0), 1)
    offs = np.concatenate([[0], np.cumsum(Kg)]).astype(np.int64)
    C = int(offs[-1])
    Cpad = ((C + WIN - 1) // WIN) * WIN
    p.Kg, p.offs, p.C, p.Cpad = Kg, offs, C, Cpad
    p.orders = orders

    gslot = np.full(N, -1, np.int64)
    for c in range(CORES):
        m = orders[c] >= 0
        gslot[orders[c][m]] = c * NSLOT + np.nonzero(m)[0]
    p.gslot = gslot

    oe = np.argsort(dst, kind="stable")
    se, sd = src[oe], dst[oe]
    starts = np.searchsorted(sd, np.arange(N))
    j_in_node = np.arange(len(sd)) - starts[sd]

    rank_of = np.full(N, -1, np.int64)
    for c in range(CORES):
        m = orders[c] >= 0
        rank_of[orders[c][m]] = np.nonzero(m)[0]

    idx_grid = np.zeros((CORES, 128, Cpad), np.int16)
    maskq = np.zeros((CORES, 128, Cpad, 4), np.float32)
    deg_row = np.ones((CORES, 1, NSLOT), np.float32)
    deg_pg = np.ones((CORES, 128, NG), np.float32)

    g_r = np.arange(NSLOT) // 128
    p_r = np.arange(NSLOT) % 128

    for c in range(CORES):
        em = (sd >= c * SH) & (sd < (c + 1) * SH)
        r_e = rank_of[sd[em]]
        cols = offs[g_r[r_e]] + j_in_node[em]
        parts = p_r[r_e]
        gs = gslot[se[em]]
        idx_grid[c, parts, cols] = (gs >> 2).astype(np.int16)
        maskq[c, parts, cols, gs & 3] = 1.0
        m = orders[c] >= 0
        r = np.nonzero(m)[0]
        node = orders[c][m]
        cols_s = offs[g_r[r]] + counts[node] - 1
        gs_s = gslot[node]
        idx_grid[c, p_r[r], cols_s] = (gs_s >> 2).astype(np.int16)
        maskq[c, p_r[r], cols_s, gs_s & 3] = 1.0
        deg_row[c, 0, r] = counts[node]
        deg_pg[c, p_r[r], g_r[r]] = counts[node]

    idxw = np.zeros((CORES, 128, Cpad * 8), np.int16)
    for c in range(CORES):
        F = idx_grid[c].T.reshape(-1)
        W16 = F.reshape(-1, 16).T
        idxw[c, 0:16] = W16
        idxw[c, 16:32] = W16

    p.idxw = idxw
    p.valid = (maskq.sum(3, keepdims=True) > 0).astype(np.float32)  # [C,128,Cpad,1]
    p.maskq = maskq.reshape(CORES, 128, Cpad, 4, 1)
    p.deg_row, p.deg_pg = deg_row, deg_pg.reshape(CORES, 128, NG, 1)

    wins = []
    for w0 in range(0, Cpad, WIN):
        ov = []
        for g in range(NG):
            a, b = max(offs[g], w0), min(offs[g + 1], w0 + WIN)
            if a < b:
                ov.append((g, int(a - w0), int(b - w0)))
        wins.append(ov)
    p.wins = wins
    return p


def build_kernel(plan):
    import concourse.bacc as bacc
    import concourse.mybir as mybir
    import concourse.tile as tile
    from concourse.masks import make_identity

    f32 = mybir.dt.float32
    i16 = mybir.dt.int16
    Alu = mybir.AluOpType
    Act = mybir.ActivationFunctionType
    AX = mybir.AxisListType

    NSLOT, NG, Cpad = plan.NSLOT, plan.NG, plan.Cpad
    GS = CORES * NSLOT
    PACKS = GS // 4
    CW = 128  # frontend chunk width

    nc = bacc.Bacc("TRN2", target_bir_lowering=False, debug=False,
                   num_devices=CORES)

    def din(name, shape, dt=f32):
        return nc.dram_tensor(name, shape, dt, kind="ExternalInput")

    xin = din("xin", [32, NSLOT])
    cin = din("cin", [5, NSLOT])
    degrow = din("degrow", [1, NSLOT])
    degpg = din("degpg", [128, NG, 1])
    idxw_d = din("idxw", [128, Cpad * 8], i16)
    mask_d = din("maskq", [128, Cpad, 4, 1])
    valid_d = din("valid", [128, Cpad, 1])
    wnames = {
        "cg_w1": [5, 32], "cg_b1": [32, 1], "cg_w2": [32, 32], "cg_b2": [32, 1],
        "cg_aw": [32, 32], "cg_ab": [32, 1], "base_imp": [32, 1],
        "ln_g": [32, 1], "ln_b": [32, 1],
        "fe_w1": [32, 64], "fe_b1": [64, 1], "fe_w2": [64, 64], "fe_b2": [64, 1],
        "gcn1_w": [64, 64], "gat_w": [64, 64], "asrc_st": [64, 2],
        "adst_st": [64, 2], "gcn2_w": [64, 32],
        "sp_w1": [32, 16], "sp_b1": [16, 1], "sp_w2": [16, 1], "sp_b2": [1, 1],
        "gcn1_b_r": [128, 1, 64], "gat_b_r": [128, 1, 64],
        "gcn2_b_r": [128, 1, 32],
    }
    for i, F in ((1, 64), (2, 64), (3, 32)):
        for nm in "gbmv":
            wnames[f"bn{i}_{nm}_r"] = [128, 1, F]
    wd = {k: din(k, s) for k, s in wnames.items()}

    svi_out = nc.dram_tensor("svi", [NG, 128], f32, kind="ExternalOutput")
    dbg = os.environ.get("KERNEL_DEBUG", "0") == "1"
    if dbg:
        dbg_x1 = nc.dram_tensor("dbg_x1", [128, NG, 64], f32, kind="ExternalOutput")
        dbg_x2 = nc.dram_tensor("dbg_x2", [128, NG, 64], f32, kind="ExternalOutput")
        dbg_x3 = nc.dram_tensor("dbg_x3", [128, NG, 32], f32, kind="ExternalOutput")

    cc1_in = nc.dram_tensor("cc1_in", [NSLOT // 4, 256], f32, kind="Internal")
    table1 = nc.dram_tensor("table1", [PACKS, 256], f32, kind="Internal",
                            addr_space="Shared")
    cc2_in = nc.dram_tensor("cc2_in", [NSLOT // 4, 320], f32, kind="Internal")
    table2 = nc.dram_tensor("table2", [PACKS, 320], f32, kind="Internal",
                            addr_space="Shared")
    cc3_in = nc.dram_tensor("cc3_in", [NSLOT // 4, 128], f32, kind="Internal")
    table3 = nc.dram_tensor("table3", [PACKS, 128], f32, kind="Internal",
                            addr_space="Shared")

    RG = [list(range(CORES))]

    with tile.TileContext(nc) as tc:
        with (
            tc.tile_pool(name="resident", bufs=1) as rp,
            tc.tile_pool(name="work", bufs=2) as wp,
            tc.tile_pool(name="psum", bufs=2, space="PSUM") as pp,
        ):
            consts = rp.tile([128, 2048], f32, tag="consts", name="consts")
            _cur = [0]

            def calloc(P, W):
                c0 = _cur[0]
                _cur[0] += W
                assert _cur[0] <= 2048
                return consts[0:P, c0:c0 + W]

            def cload(name):
                sh = wd[name].shape
                P = sh[0]
                Wn = int(np.prod(sh[1:]))
                sl = calloc(P, Wn)
                nc.sync.dma_start(sl, wd[name].ap().rearrange(
                    {2: "a b -> a b", 3: "a b c -> a (b c)",
                     4: "a b c d -> a (b c d)"}[len(sh)]))
                view = sl
                if len(sh) == 3:
                    view = sl.rearrange("a (b c) -> a b c", b=sh[1])
                elif len(sh) == 4:
                    view = sl.rearrange("a (b c d) -> a b c d", b=sh[1], c=sh[2])
                return view

            W = {k: cload(k) for k in wnames}

            ident = calloc(128, 128)
            make_identity(nc, ident)
            ones_col = calloc(128, 1)
            nc.vector.memset(ones_col, 1.0)
            ones_row64 = calloc(1, 64)
            nc.vector.memset(ones_row64, 1.0)

            def bn_scale_shift(i, F):
                sc = calloc(128, F).rearrange("a (b c) -> a b c", b=1)
                sh = calloc(128, F).rearrange("a (b c) -> a b c", b=1)
                t = calloc(128, F).rearrange("a (b c) -> a b c", b=1)
                nc.vector.tensor_scalar(t, W[f"bn{i}_v_r"], EPS, None, Alu.add)
                nc.vector.reciprocal(t, t)
                nc.scalar.sqrt(t, t)
                nc.vector.tensor_tensor(out=sc, in0=W[f"bn{i}_g_r"], in1=t,
                                        op=Alu.mult)
                nc.vector.tensor_tensor(out=t, in0=W[f"bn{i}_m_r"], in1=sc,
                                        op=Alu.mult)
                nc.vector.tensor_tensor(out=sh, in0=W[f"bn{i}_b_r"], in1=t,
                                        op=Alu.subtract)
                return sc, sh

            inv_pg = rp.tile([128, NG, 1], f32, tag="invpg", name="invpg")
            nc.sync.dma_start(inv_pg[:], degpg.ap())
            nc.vector.reciprocal(inv_pg[:], inv_pg[:])
            nc.scalar.sqrt(inv_pg[:], inv_pg[:])

            s_mask = rp.tile([128, Cpad, 4, 1], f32, tag="mask", name="mask")
            nc.sync.dma_start(s_mask[:], mask_d.ap())
            s_valid = rp.tile([128, Cpad, 1], f32, tag="valid", name="valid")
            nc.sync.dma_start(s_valid[:], valid_d.ap())

            def mm(lhsT_ap, rhs_ap, m, w, tag="mmps"):
                ps = pp.tile([m, w], f32, tag=tag, name=tag)
                nc.tensor.matmul(ps[:], lhsT_ap, rhs_ap, start=True, stop=True)
                return ps

            def bcast_row(row_ap, F, w):
                return mm(ones_row64[:, :F], row_ap, F, w, tag="bcps")

            def psum_sum32(rhs_ap, w):
                return mm(ones_col[0:32, :], rhs_ap, 1, w, tag="s32ps")

            def transpose_to(in_ap, k, m, tag="tpps"):
                ps = pp.tile([m, k], f32, tag=tag, name=tag)
                nc.tensor.transpose(ps[:], in_ap, ident[0:k, 0:k])
                return ps

            chunks = []
            c0 = 0
            while c0 < NSLOT:
                w = min(CW, NSLOT - c0)
                chunks.append((c0, w))
                c0 += w

            # group g's acc is complete once the window covering column
            # offs[g+1]-1 has been accumulated; emit per-group epilogues
            # (bn+relu, next table build, head) right there so vector/PE
            # work overlaps the gpsimd-bound gather descgen of later windows
            done_after = {}
            for g in range(NG):
                wi = (int(plan.offs[g + 1]) - 1) // WIN
                done_after.setdefault(wi, []).append(g)

            # frontend/table scratch layout (cols of a [128, 3328] tile)
            FOFF = dict(xc=0, ctc=128, ce1=256, ce2=384, ez=512, att=640,
                        xg=768, sq=896, xn=1024, f1=1152, x0=1280, h1=1408,
                        xt=1536, stk=1664, h2=1792, adc=1920, st0=2048,
                        st1=2176, st2=2304, st3=2432, st4=2560, st5=2688,
                        st6=2816, pk=2944)

            def scf_tile():
                scf = wp.tile([128, 3328], f32, tag="scf", name="scf")
                return scf

            def build_table_chunk(scf, hname, F, w, c0, pk_cols, cc_dst):
                h_sb = scf[0:F, FOFF[hname]:FOFF[hname] + w]
                pk = scf[:, FOFF["pk"]:FOFF["pk"] + pk_cols]
                if pk_cols != 4 * F:
                    nc.vector.memset(pk, 0.0)
                q4 = w // 4
                for q in range(4):
                    tp = transpose_to(h_sb[:, q::4], F, q4)
                    nc.vector.tensor_copy(
                        out=pk[0:q4, q * (pk_cols // 4):q * (pk_cols // 4) + F],
                        in_=tp[:])
                nc.sync.dma_start(cc_dst.ap()[c0 // 4:c0 // 4 + q4, :],
                                  pk[0:q4, :])

            # ---------------- frontend + table1 ----------------
            for (c0, w) in chunks:
                scf = scf_tile()

                def S(nm, P, ww=None):
                    return scf[0:P, FOFF[nm]:FOFF[nm] + (ww or w)]

                xc = S("xc", 32)
                nc.sync.dma_start(xc, xin.ap()[:, c0:c0 + w])
                ctc = S("ctc", 5)
                nc.sync.dma_start(ctc, cin.ap()[:, c0:c0 + w])

                ps = mm(W["cg_w1"], ctc, 32, w)
                ce1 = S("ce1", 32)
                nc.scalar.activation(ce1, ps[:], Act.Relu, bias=W["cg_b1"])
                ps = mm(W["cg_w2"], ce1, 32, w)
                ce2 = S("ce2", 32)
                nc.vector.tensor_scalar(ce2, ps[:], W["cg_b2"], None, Alu.add)
                ps = mm(W["cg_aw"], ce2, 32, w)
                ez = S("ez", 32)
                nc.scalar.activation(ez, ps[:], Act.Exp, bias=W["cg_ab"])
                psS = psum_sum32(ez, w)
                rS = S("st0", 1)
                nc.vector.reciprocal(rS, psS[:])
                psb = bcast_row(rS, 32, w)
                att = S("att", 32)
                nc.vector.tensor_tensor(out=att, in0=ez, in1=psb[:], op=Alu.mult)
                xg = S("xg", 32)
                nc.vector.scalar_tensor_tensor(out=xg, in0=att,
                                               scalar=W["base_imp"], in1=xc,
                                               op0=Alu.mult, op1=Alu.mult)
                ps1 = psum_sum32(xg, w)
                sq = S("sq", 32)
                nc.vector.tensor_tensor(out=sq, in0=xg, in1=xg, op=Alu.mult)
                ps2 = psum_sum32(sq, w)
                mu = S("st1", 1)
                nc.vector.tensor_scalar(mu, ps1[:], 1.0 / 32, None, Alu.mult)
                var = S("st2", 1)
                nc.vector.tensor_scalar(var, ps2[:], 1.0 / 32, None, Alu.mult)
                musq = S("st3", 1)
                nc.vector.tensor_tensor(out=musq, in0=mu, in1=mu, op=Alu.mult)
                nc.vector.tensor_tensor(out=var, in0=var, in1=musq,
                                        op=Alu.subtract)
                nc.vector.tensor_scalar(var, var, EPS, None, Alu.add)
                nc.vector.reciprocal(var, var)
                rstd = S("st4", 1)
                nc.scalar.sqrt(rstd, var)
                mrs = S("st5", 1)
                nc.vector.tensor_tensor(out=mrs, in0=mu, in1=rstd, op=Alu.mult)
                psA = bcast_row(rstd, 32, w)
                xn = S("xn", 32)
                nc.vector.tensor_tensor(out=xn, in0=xg, in1=psA[:], op=Alu.mult)
                psB = bcast_row(mrs, 32, w)
                nc.vector.tensor_tensor(out=xn, in0=xn, in1=psB[:],
                                        op=Alu.subtract)
                nc.vector.tensor_scalar(xn, xn, W["ln_g"], W["ln_b"],
                                        Alu.mult, Alu.add)
                ps = mm(W["fe_w1"], xn, 64, w)
                f1 = S("f1", 64)
                nc.scalar.activation(f1, ps[:], Act.Relu, bias=W["fe_b1"])
                ps = mm(W["fe_w2"], f1, 64, w)
                x0 = S("x0", 64)
                nc.scalar.activation(x0, ps[:], Act.Relu, bias=W["fe_b2"])
                ps = mm(W["gcn1_w"], x0, 64, w)
                invc = S("st6", 1)
                nc.sync.dma_start(invc, degrow.ap()[:, c0:c0 + w])
                nc.vector.reciprocal(invc, invc)
                nc.scalar.sqrt(invc, invc)
                psI = bcast_row(invc, 64, w)
                h1 = S("h1", 64)
                nc.scalar.activation(h1, ps[:], Act.Copy)
                nc.vector.tensor_tensor(out=h1, in0=h1, in1=psI[:],
                                        op=Alu.mult)
                build_table_chunk(scf, "h1", 64, w, c0, 256, cc1_in)

            nc.gpsimd.collective_compute(
                "AllGather", Alu.bypass, replica_groups=RG,
                ins=[cc1_in.ap()], outs=[table1.ap()])

            # ---------------- generic GCN conv ----------------
            def gcn_conv(table, F, acc, after_group=None):
                nc.vector.memset(acc, 0.0)
                for wi, ov in enumerate(plan.wins):
                    c0 = wi * WIN
                    idxt = wp.tile([128, WIN * 8], i16, tag="idxt", name="idxt")
                    nc.sync.dma_start(idxt[:],
                                      idxw_d.ap()[:, c0 * 8:(c0 + WIN) * 8])
                    gt = wp.tile([128, WIN, 4 * F], f32, tag="gt", name="gt")
                    nc.gpsimd.dma_gather(gt[:], table.ap(), idxt[:],
                                         128 * WIN, 128 * WIN, 4 * F)
                    mq = wp.tile([128, WIN, 4, F], f32, tag="mq", name="mq")
                    nc.vector.tensor_tensor(
                        out=mq[:],
                        in0=gt[:].rearrange("p w (q f) -> p w q f", q=4),
                        in1=s_mask[:, c0:c0 + WIN].to_broadcast(
                            [128, WIN, 4, F]),
                        op=Alu.mult)
                    scw = wp.tile([128, 512], f32, tag="scw", name="scw")
                    for oi, (g, a, b) in enumerate(ov):
                        red = scw[:, 322 + 64 * (oi % 2):322 + 64 * (oi % 2) + F]
                        nc.vector.tensor_reduce(
                            out=red,
                            in_=mq[:, a:b].rearrange("p c q f -> p f c q"),
                            axis=AX.XY, op=Alu.add)
                        nc.vector.tensor_tensor(out=acc[:, g], in0=acc[:, g],
                                                in1=red, op=Alu.add)
                    if after_group is not None:
                        for g in done_after.get(wi, []):
                            after_group(g)

            def post_gcn_group(acc, x, g, F, bias_r, sc, sh):
                xg = x[:, g:g + 1, 0:F]
                nc.vector.tensor_tensor(
                    out=xg, in0=acc[:, g:g + 1, 0:F],
                    in1=inv_pg[:, g:g + 1].to_broadcast([128, 1, F]),
                    op=Alu.mult)
                nc.vector.tensor_tensor(
                    out=xg, in0=xg, in1=bias_r.to_broadcast([128, 1, F]),
                    op=Alu.add)
                nc.vector.tensor_tensor(
                    out=xg, in0=xg, in1=sc.to_broadcast([128, 1, F]),
                    op=Alu.mult)
                nc.vector.tensor_tensor(
                    out=xg, in0=xg, in1=sh.to_broadcast([128, 1, F]),
                    op=Alu.add)
                nc.scalar.activation(xg, xg, Act.Relu)

            # ---------------- conv1 + interleaved table2 build ----------
            ad_pg = rp.tile([128, NG, 2], f32, tag="adpg", name="adpg")
            sc1, sh1 = bn_scale_shift(1, 64)
            x1 = rp.tile([128, NG, 64], f32, tag="bigB", name="x1")

            def fe2_chunk(g):
                scf = scf_tile()
                xt = scf[0:64, FOFF["xt"]:FOFF["xt"] + 128]
                tp = transpose_to(x1[:, g], 128, 64)
                nc.vector.tensor_copy(out=xt, in_=tp[:])
                psh = mm(W["gat_w"], xt, 64, 128)
                stk = scf[0:66, FOFF["stk"]:FOFF["stk"] + 128]
                nc.vector.tensor_copy(out=stk[0:64], in_=psh[:])
                psa = mm(W["asrc_st"], stk[0:64], 2, 128)
                nc.vector.tensor_copy(out=stk[64:66], in_=psa[:])
                psd = mm(W["adst_st"], stk[0:64], 2, 128)
                adc = scf[0:2, FOFF["adc"]:FOFF["adc"] + 128]
                nc.vector.tensor_copy(out=adc, in_=psd[:])
                tpd = transpose_to(adc, 2, 128)
                nc.vector.tensor_copy(out=ad_pg[:, g], in_=tpd[:])
                build_table_chunk(scf, "stk", 66, 128, g * 128, 320, cc2_in)

            acc1 = rp.tile([128, NG, 64], f32, tag="bigA", name="acc1")

            def post1_and_fe2(g):
                post_gcn_group(acc1, x1, g, 64, W["gcn1_b_r"], sc1, sh1)
                fe2_chunk(g)

            gcn_conv(table1, 64, acc1[:], after_group=post1_and_fe2)
            if dbg:
                nc.sync.dma_start(dbg_x1.ap(), x1[:])

            # expand ad_pg to per-column layout (overlaps AllGather)
            ad_col = rp.tile([128, Cpad, 2], f32, tag="adcol", name="adcol")
            for g in range(NG):
                a, b = int(plan.offs[g]), int(plan.offs[g + 1])
                nc.vector.tensor_copy(
                    out=ad_col[:, a:b],
                    in_=ad_pg[:, g:g + 1].to_broadcast([128, b - a, 2]))

            nc.gpsimd.collective_compute(
                "AllGather", Alu.bypass, replica_groups=RG,
                ins=[cc2_in.ap()], outs=[table2.ap()])

            # ---------------- GAT conv + interleaved table3 build --------
            N_pg = rp.tile([128, NG, 64], f32, tag="bigA", name="N_pg")
            nc.vector.memset(N_pg[:], 0.0)
            S_pg = rp.tile([128, NG, 2], f32, tag="spg", name="S_pg")
            nc.vector.memset(S_pg[:], 0.0)
            sc2, sh2 = bn_scale_shift(2, 64)
            rS2 = rp.tile([128, NG, 2], f32, tag="rs2", name="rS2")
            x2 = rp.tile([128, NG, 64], f32, tag="bigB", name="x2")

            def fe3_chunk(g):
                scf = scf_tile()
                xt = scf[0:64, FOFF["xt"]:FOFF["xt"] + 128]
                tp = transpose_to(x2[:, g], 128, 64)
                nc.vector.tensor_copy(out=xt, in_=tp[:])
                ps = mm(W["gcn2_w"], xt, 32, 128)
                invc = scf[0:1, FOFF["st6"]:FOFF["st6"] + 128]
                nc.sync.dma_start(invc, degrow.ap()[:, g * 128:(g + 1) * 128])
                nc.vector.reciprocal(invc, invc)
                nc.scalar.sqrt(invc, invc)
                psI = bcast_row(invc, 32, 128)
                h2 = scf[0:32, FOFF["h2"]:FOFF["h2"] + 128]
                nc.scalar.activation(h2, ps[:], Act.Copy)
                nc.vector.tensor_tensor(out=h2, in0=h2, in1=psI[:],
                                        op=Alu.mult)
                build_table_chunk(scf, "h2", 32, 128, g * 128, 128, cc3_in)

            def post2_and_fe3(g):
                rg = rS2[:, g:g + 1]
                nc.vector.tensor_scalar(rg, S_pg[:, g:g + 1], 1e-16, None,
                                        Alu.add)
                nc.vector.reciprocal(rg, rg)
                for h in range(2):
                    nc.vector.tensor_tensor(
                        out=x2[:, g:g + 1, 32 * h:32 * h + 32],
                        in0=N_pg[:, g:g + 1, 32 * h:32 * h + 32],
                        in1=rg[:, :, h:h + 1].to_broadcast([128, 1, 32]),
                        op=Alu.mult)
                xg = x2[:, g:g + 1]
                nc.vector.tensor_tensor(
                    out=xg, in0=xg,
                    in1=W["gat_b_r"].to_broadcast([128, 1, 64]), op=Alu.add)
                nc.vector.tensor_tensor(
                    out=xg, in0=xg, in1=sc2.to_broadcast([128, 1, 64]),
                    op=Alu.mult)
                nc.vector.tensor_tensor(
                    out=xg, in0=xg, in1=sh2.to_broadcast([128, 1, 64]),
                    op=Alu.add)
                nc.scalar.activation(xg, xg, Act.Relu)
                fe3_chunk(g)

            for wi, ov in enumerate(plan.wins):
                c0 = wi * WIN
                idxt = wp.tile([128, WIN * 8], i16, tag="idxt", name="idxt")
                nc.sync.dma_start(idxt[:], idxw_d.ap()[:, c0 * 8:(c0 + WIN) * 8])
                gt = wp.tile([128, WIN, 4, 80], f32, tag="gt", name="gt")
                nc.gpsimd.dma_gather(
                    gt[:].rearrange("p w q f -> p w (q f)"), table2.ap(),
                    idxt[:], 128 * WIN, 128 * WIN, 320)
                mq = wp.tile([128, WIN, 4, 66], f32, tag="mq", name="mq")
                nc.vector.tensor_tensor(
                    out=mq[:], in0=gt[:, :, :, 0:66],
                    in1=s_mask[:, c0:c0 + WIN].to_broadcast([128, WIN, 4, 66]),
                    op=Alu.mult)
                sel = wp.tile([128, WIN, 66], f32, tag="sel", name="sel")
                nc.vector.tensor_reduce(
                    out=sel[:], in_=mq[:].rearrange("p c q f -> p c f q"),
                    axis=AX.X, op=Alu.add)
                scw = wp.tile([128, 512], f32, tag="scw", name="scw")
                # whole-window attention math (no per-ov splitting)
                e = scw[:, 0:16].rearrange("p (c h) -> p c h", h=2)
                nc.vector.tensor_tensor(
                    out=e, in0=sel[:, :, 64:66],
                    in1=ad_col[:, c0:c0 + WIN], op=Alu.add)
                e2 = scw[:, 16:32].rearrange("p (c h) -> p c h", h=2)
                nc.vector.tensor_scalar(e2, e, 0.2, None, Alu.mult)
                nc.vector.tensor_tensor(out=e, in0=e, in1=e2, op=Alu.max)
                nc.scalar.activation(e, e, Act.Exp)
                nc.vector.tensor_tensor(
                    out=e, in0=e,
                    in1=s_valid[:, c0:c0 + WIN].to_broadcast([128, WIN, 2]),
                    op=Alu.mult)
                nm = wp.tile([128, WIN, 2, 32], f32, tag="nm", name="nm")
                e4 = scw[:, 0:16].rearrange("p (c h f) -> p c h f",
                                            h=2, f=1)
                nc.vector.tensor_tensor(
                    out=nm[:],
                    in0=sel[:, :, 0:64].rearrange("p c (h f) -> p c h f", h=2),
                    in1=e4.to_broadcast([128, WIN, 2, 32]),
                    op=Alu.mult)
                for (g, a, b) in ov:
                    ncol = b - a
                    red2 = scw[:, 32:34]
                    nc.vector.tensor_reduce(
                        out=red2, in_=e[:, a:b].rearrange("p c h -> p h c"),
                        axis=AX.X, op=Alu.add)
                    nc.vector.tensor_tensor(out=S_pg[:, g], in0=S_pg[:, g],
                                            in1=red2, op=Alu.add)
                    redh = scw[:, 34:98]
                    nc.vector.tensor_reduce(
                        out=redh.rearrange("p (h f) -> p h f", h=2),
                        in_=nm[:, a:b].rearrange("p c h f -> p h f c"),
                        axis=AX.X, op=Alu.add)
                    nc.vector.tensor_tensor(out=N_pg[:, g], in0=N_pg[:, g],
                                            in1=redh, op=Alu.add)
                for g in done_after.get(wi, []):
                    post2_and_fe3(g)

            if dbg:
                nc.sync.dma_start(dbg_x2.ap(), x2[:])

            nc.gpsimd.collective_compute(
                "AllGather", Alu.bypass, replica_groups=RG,
                ins=[cc3_in.ap()], outs=[table3.ap()])

            # ---------------- conv3 + interleaved head ----------------
            sc3, sh3 = bn_scale_shift(3, 32)
            x3 = rp.tile([128, NG, 64], f32, tag="bigB", name="x3")
            acc3 = rp.tile([128, NG, 64], f32, tag="bigA", name="acc3")

            def head_chunk(g):
                scw = wp.tile([128, 512], f32, tag="scw", name="scw")
                tp = transpose_to(x3[:, g, 0:32], 128, 32)
                xh = scw[0:32, 0:128]
                nc.vector.tensor_copy(out=xh, in_=tp[:])
                ps = mm(W["sp_w1"], xh, 16, 128)
                hh = scw[0:16, 128:256]
                nc.scalar.activation(hh, ps[:], Act.Relu, bias=W["sp_b1"])
                ps = mm(W["sp_w2"], hh, 1, 128)
                sv = scw[0:1, 256:384]
                nc.scalar.activation(sv, ps[:], Act.Sigmoid, bias=W["sp_b2"])
                nc.sync.dma_start(svi_out.ap()[g:g + 1, :], sv)

            def post3_and_head(g):
                post_gcn_group(acc3, x3, g, 32, W["gcn2_b_r"], sc3, sh3)
                head_chunk(g)

            gcn_conv(table3, 32, acc3[:, :, 0:32], after_group=post3_and_head)
            if dbg:
                nc.sync.dma_start(dbg_x3.ap(), x3[:, :, 0:32])

    nc.compile()
    return nc


def _make_inputs(plan, inputs):
    NSLOT = plan.NSLOT
    xf = inputs["accessibility_features"].astype(np.float32)
    cf = inputs["context_features"].astype(np.float32)

    def col(a):
        return np.ascontiguousarray(np.asarray(a, np.float32).reshape(-1, 1))

    def rep(a, shape):
        return np.ascontiguousarray(
            np.broadcast_to(np.asarray(a, np.float32), shape))

    common = {
        "cg_w1": np.asarray(inputs["cg_w1"], np.float32),
        "cg_b1": col(inputs["cg_b1"]),
        "cg_w2": np.asarray(inputs["cg_w2"], np.float32),
        "cg_b2": col(inputs["cg_b2"]),
        "cg_aw": np.asarray(inputs["cg_aw"], np.float32),
        "cg_ab": col(inputs["cg_ab"]),
        "base_imp": col(inputs["base_imp"]),
        "ln_g": col(inputs["ln_g"]), "ln_b": col(inputs["ln_b"]),
        "fe_w1": np.asarray(inputs["fe_w1"], np.float32),
        "fe_b1": col(inputs["fe_b1"]),
        "fe_w2": np.asarray(inputs["fe_w2"], np.float32),
        "fe_b2": col(inputs["fe_b2"]),
        "gcn1_w": np.asarray(inputs["gcn1_w"], np.float32),
        "gat_w": np.asarray(inputs["gat_w"], np.float32),
        "gcn2_w": np.asarray(inputs["gcn2_w"], np.float32),
        "sp_w1": np.asarray(inputs["sp_w1"], np.float32),
        "sp_b1": col(inputs["sp_b1"]),
        "sp_w2": np.asarray(inputs["sp_w2"], np.float32),
        "sp_b2": col(inputs["sp_b2"]),
        "gcn1_b_r": rep(inputs["gcn1_b"][None, None, :], (128, 1, 64)),
        "gat_b_r": rep(inputs["gat_b"][None, None, :], (128, 1, 64)),
        "gcn2_b_r": rep(inputs["gcn2_b"][None, None, :], (128, 1, 32)),
    }
    asrc = np.asarray(inputs["gat_asrc"], np.float32)
    ast = np.zeros((64, 2), np.float32)
    ast[0:32, 0] = asrc[0]
    ast[32:64, 1] = asrc[1]
    common["asrc_st"] = ast
    adst = np.asarray(inputs["gat_adst"], np.float32)
    adt2 = np.zeros((64, 2), np.float32)
    adt2[0:32, 0] = adst[0]
    adt2[32:64, 1] = adst[1]
    common["adst_st"] = adt2
    for i in (1, 2, 3):
        F = 32 if i == 3 else 64
        for nm in "gbmv":
            common[f"bn{i}_{nm}_r"] = rep(
                np.asarray(inputs[f"bn{i}_{nm}"])[None, None, :], (128, 1, F))

    maps = []
    for c in range(CORES):
        o = plan.orders[c]
        m = o >= 0
        xs = np.zeros((NSLOT, 32), np.float32)
        cs = np.zeros((NSLOT, 5), np.float32)
        xs[m] = xf[o[m]]
        cs[m] = cf[o[m]]
        im = dict(common)
        im["xin"] = np.ascontiguousarray(xs.T)
        im["cin"] = np.ascontiguousarray(cs.T)
        im["degrow"] = plan.deg_row[c]
        im["degpg"] = plan.deg_pg[c]
        im["idxw"] = plan.idxw[c]
        im["maskq"] = plan.maskq[c]
        im["valid"] = plan.valid[c]
        maps.append(im)
    return maps


def run_sim(plan, nc, in_maps):
    """CoreSim single-core run (core 0) — collectives emulated by writing all
    cores' table slices is not possible; instead run with CORES small."""
    raise NotImplementedError


def kernel(**inputs):
    from concourse.bass_utils import run_bass_kernel_spmd

    edge_index = np.asarray(inputs["edge_index"])
    N = inputs["accessibility_features"].shape[0]
    plan = build_plan(edge_index, N)
    nc = build_kernel(plan)
    in_maps = _make_inputs(plan, inputs)

    trace = os.environ.get("KERNEL_TRACE", "0") == "1"
    res = run_bass_kernel_spmd(nc, in_maps, core_ids=list(range(CORES)),
                               trace=trace)
    kernel.last_result = res

    svi = np.zeros(N, np.float32)
    for c in range(CORES):
        o = plan.orders[c]
        m = o >= 0
        flat = res.results[c]["svi"].reshape(plan.NG * 128)
        svi[o[m]] = flat[np.nonzero(m)[0]]
    return svi



# revision 5
# speedup vs baseline: 1.0197x; 1.0197x over previous
"""AccessibilitySVIGNN Trainium2 kernel (8-core SPMD).

See bottom of file for entry point `kernel(**inputs)`.

Design:
- Nodes dst-sharded across 8 cores; per-core slot grid (nodes degree-sorted
  into 128-partition groups, uniform per-group column count across cores so
  one SPMD program works for all cores). Every edge incl. self-loop = 1 slot.
- Gather tables pack 4 nodes per row so an int16 pack index covers 100k nodes;
  a host-built one-hot mask selects the quadrant on the vector engine.
- GCN normalization via pre-scaled tables (1/sqrt(deg_src)) + post-scale.
- GAT via gathered [h | alpha_src] rows + online plain-exp softmax.
- Dense per-node compute feature-transposed on the tensor engine; conv tables
  exchanged with AllGather collectives.
"""

import math
import os

import numpy as np

EPS = 1e-5
CORES = 8
WIN = 8  # gather window: 8 cols x 128 partitions = 1024 idxs per dma_gather


class Plan:
    pass


def build_plan(edge_index, n_nodes):
    p = Plan()
    N = n_nodes
    src = edge_index[0].astype(np.int64)
    dst = edge_index[1].astype(np.int64)
    SH = N // CORES
    NG = math.ceil(SH / 128)
    NSLOT = NG * 128
    p.N, p.SH, p.NG, p.NSLOT = N, SH, NG, NSLOT

    counts = np.bincount(dst, minlength=N) + 1  # incl self-loop

    orders = np.full((CORES, NSLOT), -1, np.int64)
    Ks = np.zeros((CORES, NG), np.int64)
    for c in range(CORES):
        nodes = np.arange(c * SH, (c + 1) * SH)
        o = np.argsort(-counts[nodes], kind="stable")
        orders[c, :SH] = nodes[o]
        cnt_sorted = np.zeros(NSLOT, np.int64)
        cnt_sorted[:SH] = counts[nodes][o]
        Ks[c] = cnt_sorted.reshape(NG, 128).max(1)
    Kg = np.maximum(Ks.max(0), 1)
    offs = np.concatenate([[0], np.cumsum(Kg)]).astype(np.int64)
    C = int(offs[-1])
    Cpad = ((C + WIN - 1) // WIN) * WIN
    p.Kg, p.offs, p.C, p.Cpad = Kg, offs, C, Cpad
    p.orders = orders

    gslot = np.full(N, -1, np.int64)
    for c in range(CORES):
        m = orders[c] >= 0
        gslot[orders[c][m]] = c * NSLOT + np.nonzero(m)[0]
    p.gslot = gslot

    oe = np.argsort(dst, kind="stable")
    se, sd = src[oe], dst[oe]
    starts = np.searchsorted(sd, np.arange(N))
    j_in_node = np.arange(len(sd)) - starts[sd]

    rank_of = np.full(N, -1, np.int64)
    for c in range(CORES):
        m = orders[c] >= 0
        rank_of[orders[c][m]] = np.nonzero(m)[0]

    idx_grid = np.zeros((CORES, 128, Cpad), np.int16)
    maskq = np.zeros((CORES, 128, Cpad, 4), np.float32)
    deg_row = np.ones((CORES, 1, NSLOT), np.float32)
    deg_pg = np.ones((CORES, 128, NG), np.float32)

    g_r = np.arange(NSLOT) // 128
    p_r = np.arange(NSLOT) % 128

    for c in range(CORES):
        em = (sd >= c * SH) & (sd < (c + 1) * SH)
        r_e = rank_of[sd[em]]
        cols = offs[g_r[r_e]] + j_in_node[em]
        parts = p_r[r_e]
        gs = gslot[se[em]]
        idx_grid[c, parts, cols] = (gs >> 2).astype(np.int16)
        maskq[c, parts, cols, gs & 3] = 1.0
        m = orders[c] >= 0
        r = np.nonzero(m)[0]
        node = orders[c][m]
        cols_s = offs[g_r[r]] + counts[node] - 1
        gs_s = gslot[node]
        idx_grid[c, p_r[r], cols_s] = (gs_s >> 2).astype(np.int16)
        maskq[c, p_r[r], cols_s, gs_s & 3] = 1.0
        deg_row[c, 0, r] = counts[node]
        deg_pg[c, p_r[r], g_r[r]] = counts[node]

    idxw = np.zeros((CORES, 128, Cpad * 8), np.int16)
    for c in range(CORES):
        F = idx_grid[c].T.reshape(-1)
        W16 = F.reshape(-1, 16).T
        for b in range(8):
            idxw[c, b * 16:(b + 1) * 16] = W16

    p.idxw = idxw
    p.valid = (maskq.sum(3, keepdims=True) > 0).astype(np.float32)  # [C,128,Cpad,1]
    p.maskq = maskq.reshape(CORES, 128, Cpad, 4, 1)
    p.deg_row, p.deg_pg = deg_row, deg_pg.reshape(CORES, 128, NG, 1)

    wins = []
    for w0 in range(0, Cpad, WIN):
        ov = []
        for g in range(NG):
            a, b = max(offs[g], w0), min(offs[g + 1], w0 + WIN)
            if a < b:
                ov.append((g, int(a - w0), int(b - w0)))
        wins.append(ov)
    p.wins = wins
    return p


def build_kernel(plan):
    import concourse.bacc as bacc
    import concourse.mybir as mybir
    import concourse.tile as tile
    from concourse.masks import make_identity

    f32 = mybir.dt.float32
    i16 = mybir.dt.int16
    Alu = mybir.AluOpType
    Act = mybir.ActivationFunctionType
    AX = mybir.AxisListType

    NSLOT, NG, Cpad = plan.NSLOT, plan.NG, plan.Cpad
    GS = CORES * NSLOT
    PACKS = GS // 4
    CW = 128  # frontend chunk width

    nc = bacc.Bacc("TRN2", target_bir_lowering=False, debug=False,
                   num_devices=CORES, num_swdge_queues=4)

    def din(name, shape, dt=f32):
        return nc.dram_tensor(name, shape, dt, kind="ExternalInput")

    xin = din("xin", [32, NSLOT])
    cin = din("cin", [5, NSLOT])
    degrow = din("degrow", [1, NSLOT])
    degpg = din("degpg", [128, NG, 1])
    idxw_d = din("idxw", [128, Cpad * 8], i16)
    mask_d = din("maskq", [128, Cpad, 4, 1])
    valid_d = din("valid", [128, Cpad, 1])
    wnames = {
        "cg_w1": [5, 32], "cg_b1": [32, 1], "cg_w2": [32, 32], "cg_b2": [32, 1],
        "cg_aw": [32, 32], "cg_ab": [32, 1], "base_imp": [32, 1],
        "ln_g": [32, 1], "ln_b": [32, 1],
        "fe_w1": [32, 64], "fe_b1": [64, 1], "fe_w2": [64, 64], "fe_b2": [64, 1],
        "gcn1_w": [64, 64], "gat_w": [64, 64], "asrc_st": [64, 2],
        "adst_st": [64, 2], "gcn2_w": [64, 32],
        "sp_w1": [32, 16], "sp_b1": [16, 1], "sp_w2": [16, 1], "sp_b2": [1, 1],
        "gcn1_b_r": [128, 1, 64], "gat_b_r": [128, 1, 64],
        "gcn2_b_r": [128, 1, 32],
    }
    for i, F in ((1, 64), (2, 64), (3, 32)):
        for nm in "gbmv":
            wnames[f"bn{i}_{nm}_r"] = [128, 1, F]
    wd = {k: din(k, s) for k, s in wnames.items()}

    svi_out = nc.dram_tensor("svi", [NG, 128], f32, kind="ExternalOutput")
    dbg = os.environ.get("KERNEL_DEBUG", "0") == "1"
    if dbg:
        dbg_x1 = nc.dram_tensor("dbg_x1", [128, NG, 64], f32, kind="ExternalOutput")
        dbg_x2 = nc.dram_tensor("dbg_x2", [128, NG, 64], f32, kind="ExternalOutput")
        dbg_x3 = nc.dram_tensor("dbg_x3", [128, NG, 32], f32, kind="ExternalOutput")

    cc1_in = nc.dram_tensor("cc1_in", [NSLOT // 4, 256], f32, kind="Internal")
    table1 = nc.dram_tensor("table1", [PACKS, 256], f32, kind="Internal",
                            addr_space="Shared")
    cc2_in = nc.dram_tensor("cc2_in", [NSLOT // 4, 320], f32, kind="Internal")
    table2 = nc.dram_tensor("table2", [PACKS, 320], f32, kind="Internal",
                            addr_space="Shared")
    cc3_in = nc.dram_tensor("cc3_in", [NSLOT // 4, 128], f32, kind="Internal")
    table3 = nc.dram_tensor("table3", [PACKS, 128], f32, kind="Internal",
                            addr_space="Shared")

    RG = [list(range(CORES))]

    with tile.TileContext(nc) as tc:
        with (
            tc.tile_pool(name="resident", bufs=1) as rp,
            tc.tile_pool(name="work", bufs=2) as wp,
            tc.tile_pool(name="psum", bufs=2, space="PSUM") as pp,
        ):
            consts = rp.tile([128, 2048], f32, tag="consts", name="consts")
            _cur = [0]

            def calloc(P, W):
                c0 = _cur[0]
                _cur[0] += W
                assert _cur[0] <= 2048
                return consts[0:P, c0:c0 + W]

            def cload(name):
                sh = wd[name].shape
                P = sh[0]
                Wn = int(np.prod(sh[1:]))
                sl = calloc(P, Wn)
                nc.sync.dma_start(sl, wd[name].ap().rearrange(
                    {2: "a b -> a b", 3: "a b c -> a (b c)",
                     4: "a b c d -> a (b c d)"}[len(sh)]))
                view = sl
                if len(sh) == 3:
                    view = sl.rearrange("a (b c) -> a b c", b=sh[1])
                elif len(sh) == 4:
                    view = sl.rearrange("a (b c d) -> a b c d", b=sh[1], c=sh[2])
                return view

            W = {k: cload(k) for k in wnames}

            ident = calloc(128, 128)
            make_identity(nc, ident)
            ones_col = calloc(128, 1)
            nc.vector.memset(ones_col, 1.0)
            ones_row64 = calloc(1, 64)
            nc.vector.memset(ones_row64, 1.0)

            def bn_scale_shift(i, F):
                sc = calloc(128, F).rearrange("a (b c) -> a b c", b=1)
                sh = calloc(128, F).rearrange("a (b c) -> a b c", b=1)
                t = calloc(128, F).rearrange("a (b c) -> a b c", b=1)
                nc.vector.tensor_scalar(t, W[f"bn{i}_v_r"], EPS, None, Alu.add)
                nc.vector.reciprocal(t, t)
                nc.scalar.sqrt(t, t)
                nc.vector.tensor_tensor(out=sc, in0=W[f"bn{i}_g_r"], in1=t,
                                        op=Alu.mult)
                nc.vector.tensor_tensor(out=t, in0=W[f"bn{i}_m_r"], in1=sc,
                                        op=Alu.mult)
                nc.vector.tensor_tensor(out=sh, in0=W[f"bn{i}_b_r"], in1=t,
                                        op=Alu.subtract)
                return sc, sh

            inv_pg = rp.tile([128, NG, 1], f32, tag="invpg", name="invpg")
            nc.sync.dma_start(inv_pg[:], degpg.ap())
            nc.vector.reciprocal(inv_pg[:], inv_pg[:])
            nc.scalar.sqrt(inv_pg[:], inv_pg[:])

            s_mask = rp.tile([128, Cpad, 4, 1], f32, tag="mask", name="mask")
            nc.sync.dma_start(s_mask[:], mask_d.ap())
            s_valid = rp.tile([128, Cpad, 1], f32, tag="valid", name="valid")
            nc.sync.dma_start(s_valid[:], valid_d.ap())

            def mm(lhsT_ap, rhs_ap, m, w, tag="mmps"):
                ps = pp.tile([m, w], f32, tag=tag, name=tag)
                nc.tensor.matmul(ps[:], lhsT_ap, rhs_ap, start=True, stop=True)
                return ps

            def bcast_row(row_ap, F, w):
                return mm(ones_row64[:, :F], row_ap, F, w, tag="bcps")

            def psum_sum32(rhs_ap, w):
                return mm(ones_col[0:32, :], rhs_ap, 1, w, tag="s32ps")

            def transpose_to(in_ap, k, m, tag="tpps"):
                ps = pp.tile([m, k], f32, tag=tag, name=tag)
                nc.tensor.transpose(ps[:], in_ap, ident[0:k, 0:k])
                return ps

            chunks = []
            c0 = 0
            while c0 < NSLOT:
                w = min(CW, NSLOT - c0)
                chunks.append((c0, w))
                c0 += w

            # group g's acc is complete once the window covering column
            # offs[g+1]-1 has been accumulated; emit per-group epilogues
            # (bn+relu, next table build, head) right there so vector/PE
            # work overlaps the gpsimd-bound gather descgen of later windows
            done_after = {}
            for g in range(NG):
                wi = (int(plan.offs[g + 1]) - 1) // WIN
                done_after.setdefault(wi, []).append(g)

            # frontend/table scratch layout (cols of a [128, 3328] tile)
            FOFF = dict(xc=0, ctc=128, ce1=256, ce2=384, ez=512, att=640,
                        xg=768, sq=896, xn=1024, f1=1152, x0=1280, h1=1408,
                        xt=1536, stk=1664, h2=1792, adc=1920, st0=2048,
                        st1=2176, st2=2304, st3=2432, st4=2560, st5=2688,
                        st6=2816, pk=2944)

            def scf_tile():
                scf = wp.tile([128, 3328], f32, tag="scf", name="scf")
                return scf

            def build_table_chunk(scf, hname, F, w, c0, pk_cols, cc_dst):
                h_sb = scf[0:F, FOFF[hname]:FOFF[hname] + w]
                pk = scf[:, FOFF["pk"]:FOFF["pk"] + pk_cols]
                if pk_cols != 4 * F:
                    nc.vector.memset(pk, 0.0)
                q4 = w // 4
                for q in range(4):
                    tp = transpose_to(h_sb[:, q::4], F, q4)
                    nc.vector.tensor_copy(
                        out=pk[0:q4, q * (pk_cols // 4):q * (pk_cols // 4) + F],
                        in_=tp[:])
                nc.sync.dma_start(cc_dst.ap()[c0 // 4:c0 // 4 + q4, :],
                                  pk[0:q4, :])

            # ---------------- frontend + table1 ----------------
            for (c0, w) in chunks:
                scf = scf_tile()

                def S(nm, P, ww=None):
                    return scf[0:P, FOFF[nm]:FOFF[nm] + (ww or w)]

                xc = S("xc", 32)
                nc.sync.dma_start(xc, xin.ap()[:, c0:c0 + w])
                ctc = S("ctc", 5)
                nc.sync.dma_start(ctc, cin.ap()[:, c0:c0 + w])

                ps = mm(W["cg_w1"], ctc, 32, w)
                ce1 = S("ce1", 32)
                nc.scalar.activation(ce1, ps[:], Act.Relu, bias=W["cg_b1"])
                ps = mm(W["cg_w2"], ce1, 32, w)
                ce2 = S("ce2", 32)
                nc.vector.tensor_scalar(ce2, ps[:], W["cg_b2"], None, Alu.add)
                ps = mm(W["cg_aw"], ce2, 32, w)
                ez = S("ez", 32)
                nc.scalar.activation(ez, ps[:], Act.Exp, bias=W["cg_ab"])
                psS = psum_sum32(ez, w)
                rS = S("st0", 1)
                nc.vector.reciprocal(rS, psS[:])
                psb = bcast_row(rS, 32, w)
                att = S("att", 32)
                nc.vector.tensor_tensor(out=att, in0=ez, in1=psb[:], op=Alu.mult)
                xg = S("xg", 32)
                nc.vector.scalar_tensor_tensor(out=xg, in0=att,
                                               scalar=W["base_imp"], in1=xc,
                                               op0=Alu.mult, op1=Alu.mult)
                ps1 = psum_sum32(xg, w)
                sq = S("sq", 32)
                nc.vector.tensor_tensor(out=sq, in0=xg, in1=xg, op=Alu.mult)
                ps2 = psum_sum32(sq, w)
                mu = S("st1", 1)
                nc.vector.tensor_scalar(mu, ps1[:], 1.0 / 32, None, Alu.mult)
                var = S("st2", 1)
                nc.vector.tensor_scalar(var, ps2[:], 1.0 / 32, None, Alu.mult)
                musq = S("st3", 1)
                nc.vector.tensor_tensor(out=musq, in0=mu, in1=mu, op=Alu.mult)
                nc.vector.tensor_tensor(out=var, in0=var, in1=musq,
                                        op=Alu.subtract)
                nc.vector.tensor_scalar(var, var, EPS, None, Alu.add)
                nc.vector.reciprocal(var, var)
                rstd = S("st4", 1)
                nc.scalar.sqrt(rstd, var)
                mrs = S("st5", 1)
                nc.vector.tensor_tensor(out=mrs, in0=mu, in1=rstd, op=Alu.mult)
                psA = bcast_row(rstd, 32, w)
                xn = S("xn", 32)
                nc.vector.tensor_tensor(out=xn, in0=xg, in1=psA[:], op=Alu.mult)
                psB = bcast_row(mrs, 32, w)
                nc.vector.tensor_tensor(out=xn, in0=xn, in1=psB[:],
                                        op=Alu.subtract)
                nc.vector.tensor_scalar(xn, xn, W["ln_g"], W["ln_b"],
                                        Alu.mult, Alu.add)
                ps = mm(W["fe_w1"], xn, 64, w)
                f1 = S("f1", 64)
                nc.scalar.activation(f1, ps[:], Act.Relu, bias=W["fe_b1"])
                ps = mm(W["fe_w2"], f1, 64, w)
                x0 = S("x0", 64)
                nc.scalar.activation(x0, ps[:], Act.Relu, bias=W["fe_b2"])
                ps = mm(W["gcn1_w"], x0, 64, w)
                invc = S("st6", 1)
                nc.sync.dma_start(invc, degrow.ap()[:, c0:c0 + w])
                nc.vector.reciprocal(invc, invc)
                nc.scalar.sqrt(invc, invc)
                psI = bcast_row(invc, 64, w)
                h1 = S("h1", 64)
                nc.scalar.activation(h1, ps[:], Act.Copy)
                nc.vector.tensor_tensor(out=h1, in0=h1, in1=psI[:],
                                        op=Alu.mult)
                build_table_chunk(scf, "h1", 64, w, c0, 256, cc1_in)

            nc.gpsimd.collective_compute(
                "AllGather", Alu.bypass, replica_groups=RG,
                ins=[cc1_in.ap()], outs=[table1.ap()])

            # ---------------- generic GCN conv ----------------
            def gcn_conv(table, F, acc, after_group=None):
                nc.vector.memset(acc, 0.0)
                for wi, ov in enumerate(plan.wins):
                    c0 = wi * WIN
                    idxt = wp.tile([128, WIN * 8], i16, tag="idxt", name="idxt")
                    nc.sync.dma_start(idxt[:],
                                      idxw_d.ap()[:, c0 * 8:(c0 + WIN) * 8])
                    gt = wp.tile([128, WIN, 4 * F], f32, tag="gt", name="gt")
                    nc.gpsimd.dma_gather(gt[:], table.ap(), idxt[:],
                                         128 * WIN, 128 * WIN, 4 * F,
                                         queue_num=wi % 4)
                    mq = wp.tile([128, WIN, 4, F], f32, tag="mq", name="mq")
                    nc.vector.tensor_tensor(
                        out=mq[:],
                        in0=gt[:].rearrange("p w (q f) -> p w q f", q=4),
                        in1=s_mask[:, c0:c0 + WIN].to_broadcast(
                            [128, WIN, 4, F]),
                        op=Alu.mult)
                    scw = wp.tile([128, 512], f32, tag="scw", name="scw")
                    for oi, (g, a, b) in enumerate(ov):
                        red = scw[:, 322 + 64 * (oi % 2):322 + 64 * (oi % 2) + F]
                        nc.vector.tensor_reduce(
                            out=red,
                            in_=mq[:, a:b].rearrange("p c q f -> p f c q"),
                            axis=AX.XY, op=Alu.add)
                        nc.vector.tensor_tensor(out=acc[:, g], in0=acc[:, g],
                                                in1=red, op=Alu.add)
                    if after_group is not None:
                        for g in done_after.get(wi, []):
                            after_group(g)

            def post_gcn_group(acc, x, g, F, bias_r, sc, sh):
                xg = x[:, g:g + 1, 0:F]
                nc.vector.tensor_tensor(
                    out=xg, in0=acc[:, g:g + 1, 0:F],
                    in1=inv_pg[:, g:g + 1].to_broadcast([128, 1, F]),
                    op=Alu.mult)
                nc.vector.tensor_tensor(
                    out=xg, in0=xg, in1=bias_r.to_broadcast([128, 1, F]),
                    op=Alu.add)
                nc.vector.tensor_tensor(
                    out=xg, in0=xg, in1=sc.to_broadcast([128, 1, F]),
                    op=Alu.mult)
                nc.vector.tensor_tensor(
                    out=xg, in0=xg, in1=sh.to_broadcast([128, 1, F]),
                    op=Alu.add)
                nc.scalar.activation(xg, xg, Act.Relu)

            # ---------------- conv1 + interleaved table2 build ----------
            ad_pg = rp.tile([128, NG, 2], f32, tag="adpg", name="adpg")
            sc1, sh1 = bn_scale_shift(1, 64)
            x1 = rp.tile([128, NG, 64], f32, tag="bigB", name="x1")

            def fe2_chunk(g):
                scf = scf_tile()
                xt = scf[0:64, FOFF["xt"]:FOFF["xt"] + 128]
                tp = transpose_to(x1[:, g], 128, 64)
                nc.vector.tensor_copy(out=xt, in_=tp[:])
                psh = mm(W["gat_w"], xt, 64, 128)
                stk = scf[0:66, FOFF["stk"]:FOFF["stk"] + 128]
                nc.vector.tensor_copy(out=stk[0:64], in_=psh[:])
                psa = mm(W["asrc_st"], stk[0:64], 2, 128)
                nc.vector.tensor_copy(out=stk[64:66], in_=psa[:])
                psd = mm(W["adst_st"], stk[0:64], 2, 128)
                adc = scf[0:2, FOFF["adc"]:FOFF["adc"] + 128]
                nc.vector.tensor_copy(out=adc, in_=psd[:])
                tpd = transpose_to(adc, 2, 128)
                nc.vector.tensor_copy(out=ad_pg[:, g], in_=tpd[:])
                build_table_chunk(scf, "stk", 66, 128, g * 128, 320, cc2_in)

            acc1 = rp.tile([128, NG, 64], f32, tag="bigA", name="acc1")

            def post1_and_fe2(g):
                post_gcn_group(acc1, x1, g, 64, W["gcn1_b_r"], sc1, sh1)
                fe2_chunk(g)

            gcn_conv(table1, 64, acc1[:], after_group=post1_and_fe2)
            if dbg:
                nc.sync.dma_start(dbg_x1.ap(), x1[:])

            # expand ad_pg to per-column layout (overlaps AllGather)
            ad_col = rp.tile([128, Cpad, 2], f32, tag="adcol", name="adcol")
            for g in range(NG):
                a, b = int(plan.offs[g]), int(plan.offs[g + 1])
                nc.vector.tensor_copy(
                    out=ad_col[:, a:b],
                    in_=ad_pg[:, g:g + 1].to_broadcast([128, b - a, 2]))

            nc.gpsimd.collective_compute(
                "AllGather", Alu.bypass, replica_groups=RG,
                ins=[cc2_in.ap()], outs=[table2.ap()])

            # ---------------- GAT conv + interleaved table3 build --------
            N_pg = rp.tile([128, NG, 64], f32, tag="bigA", name="N_pg")
            nc.vector.memset(N_pg[:], 0.0)
            S_pg = rp.tile([128, NG, 2], f32, tag="spg", name="S_pg")
            nc.vector.memset(S_pg[:], 0.0)
            sc2, sh2 = bn_scale_shift(2, 64)
            rS2 = rp.tile([128, NG, 2], f32, tag="rs2", name="rS2")
            x2 = rp.tile([128, NG, 64], f32, tag="bigB", name="x2")

            def fe3_chunk(g):
                scf = scf_tile()
                xt = scf[0:64, FOFF["xt"]:FOFF["xt"] + 128]
                tp = transpose_to(x2[:, g], 128, 64)
                nc.vector.tensor_copy(out=xt, in_=tp[:])
                ps = mm(W["gcn2_w"], xt, 32, 128)
                invc = scf[0:1, FOFF["st6"]:FOFF["st6"] + 128]
                nc.sync.dma_start(invc, degrow.ap()[:, g * 128:(g + 1) * 128])
                nc.vector.reciprocal(invc, invc)
                nc.scalar.sqrt(invc, invc)
                psI = bcast_row(invc, 32, 128)
                h2 = scf[0:32, FOFF["h2"]:FOFF["h2"] + 128]
                nc.scalar.activation(h2, ps[:], Act.Copy)
                nc.vector.tensor_tensor(out=h2, in0=h2, in1=psI[:],
                                        op=Alu.mult)
                build_table_chunk(scf, "h2", 32, 128, g * 128, 128, cc3_in)

            def post2_and_fe3(g):
                rg = rS2[:, g:g + 1]
                nc.vector.tensor_scalar(rg, S_pg[:, g:g + 1], 1e-16, None,
                                        Alu.add)
                nc.vector.reciprocal(rg, rg)
                for h in range(2):
                    nc.vector.tensor_tensor(
                        out=x2[:, g:g + 1, 32 * h:32 * h + 32],
                        in0=N_pg[:, g:g + 1, 32 * h:32 * h + 32],
                        in1=rg[:, :, h:h + 1].to_broadcast([128, 1, 32]),
                        op=Alu.mult)
                xg = x2[:, g:g + 1]
                nc.vector.tensor_tensor(
                    out=xg, in0=xg,
                    in1=W["gat_b_r"].to_broadcast([128, 1, 64]), op=Alu.add)
                nc.vector.tensor_tensor(
                    out=xg, in0=xg, in1=sc2.to_broadcast([128, 1, 64]),
                    op=Alu.mult)
                nc.vector.tensor_tensor(
                    out=xg, in0=xg, in1=sh2.to_broadcast([128, 1, 64]),
                    op=Alu.add)
                nc.scalar.activation(xg, xg, Act.Relu)
                fe3_chunk(g)

            for wi, ov in enumerate(plan.wins):
                c0 = wi * WIN
                idxt = wp.tile([128, WIN * 8], i16, tag="idxt", name="idxt")
                nc.sync.dma_start(idxt[:], idxw_d.ap()[:, c0 * 8:(c0 + WIN) * 8])
                gt = wp.tile([128, WIN, 4, 80], f32, tag="gt", name="gt")
                nc.gpsimd.dma_gather(
                    gt[:].rearrange("p w q f -> p w (q f)"), table2.ap(),
                    idxt[:], 128 * WIN, 128 * WIN, 320,
                    queue_num=wi % 4)
                mq = wp.tile([128, WIN, 4, 66], f32, tag="mq", name="mq")
                nc.vector.tensor_tensor(
                    out=mq[:], in0=gt[:, :, :, 0:66],
                    in1=s_mask[:, c0:c0 + WIN].to_broadcast([128, WIN, 4, 66]),
                    op=Alu.mult)
                sel = wp.tile([128, WIN, 66], f32, tag="sel", name="sel")
                nc.vector.tensor_reduce(
                    out=sel[:], in_=mq[:].rearrange("p c q f -> p c f q"),
                    axis=AX.X, op=Alu.add)
                scw = wp.tile([128, 512], f32, tag="scw", name="scw")
                # whole-window attention math (no per-ov splitting)
                e = scw[:, 0:16].rearrange("p (c h) -> p c h", h=2)
                nc.vector.tensor_tensor(
                    out=e, in0=sel[:, :, 64:66],
                    in1=ad_col[:, c0:c0 + WIN], op=Alu.add)
                e2 = scw[:, 16:32].rearrange("p (c h) -> p c h", h=2)
                nc.vector.tensor_scalar(e2, e, 0.2, None, Alu.mult)
                nc.vector.tensor_tensor(out=e, in0=e, in1=e2, op=Alu.max)
                nc.scalar.activation(e, e, Act.Exp)
                nc.vector.tensor_tensor(
                    out=e, in0=e,
                    in1=s_valid[:, c0:c0 + WIN].to_broadcast([128, WIN, 2]),
                    op=Alu.mult)
                nm = wp.tile([128, WIN, 2, 32], f32, tag="nm", name="nm")
                e4 = scw[:, 0:16].rearrange("p (c h f) -> p c h f",
                                            h=2, f=1)
                nc.vector.tensor_tensor(
                    out=nm[:],
                    in0=sel[:, :, 0:64].rearrange("p c (h f) -> p c h f", h=2),
                    in1=e4.to_broadcast([128, WIN, 2, 32]),
                    op=Alu.mult)
                for (g, a, b) in ov:
                    ncol = b - a
                    red2 = scw[:, 32:34]
                    nc.vector.tensor_reduce(
                        out=red2, in_=e[:, a:b].rearrange("p c h -> p h c"),
                        axis=AX.X, op=Alu.add)
                    nc.vector.tensor_tensor(out=S_pg[:, g], in0=S_pg[:, g],
                                            in1=red2, op=Alu.add)
                    redh = scw[:, 34:98]
                    nc.vector.tensor_reduce(
                        out=redh.rearrange("p (h f) -> p h f", h=2),
                        in_=nm[:, a:b].rearrange("p c h f -> p h f c"),
                        axis=AX.X, op=Alu.add)
                    nc.vector.tensor_tensor(out=N_pg[:, g], in0=N_pg[:, g],
                                            in1=redh, op=Alu.add)
                for g in done_after.get(wi, []):
                    post2_and_fe3(g)

            if dbg:
                nc.sync.dma_start(dbg_x2.ap(), x2[:])

            nc.gpsimd.collective_compute(
                "AllGather", Alu.bypass, replica_groups=RG,
                ins=[cc3_in.ap()], outs=[table3.ap()])

            # ---------------- conv3 + interleaved head ----------------
            sc3, sh3 = bn_scale_shift(3, 32)
            x3 = rp.tile([128, NG, 64], f32, tag="bigB", name="x3")
            acc3 = rp.tile([128, NG, 64], f32, tag="bigA", name="acc3")

            def head_chunk(g):
                scw = wp.tile([128, 512], f32, tag="scw", name="scw")
                tp = transpose_to(x3[:, g, 0:32], 128, 32)
                xh = scw[0:32, 0:128]
                nc.vector.tensor_copy(out=xh, in_=tp[:])
                ps = mm(W["sp_w1"], xh, 16, 128)
                hh = scw[0:16, 128:256]
                nc.scalar.activation(hh, ps[:], Act.Relu, bias=W["sp_b1"])
                ps = mm(W["sp_w2"], hh, 1, 128)
                sv = scw[0:1, 256:384]
                nc.scalar.activation(sv, ps[:], Act.Sigmoid, bias=W["sp_b2"])
                nc.sync.dma_start(svi_out.ap()[g:g + 1, :], sv)

            def post3_and_head(g):
                post_gcn_group(acc3, x3, g, 32, W["gcn2_b_r"], sc3, sh3)
                head_chunk(g)

            gcn_conv(table3, 32, acc3[:, :, 0:32], after_group=post3_and_head)
            if dbg:
                nc.sync.dma_start(dbg_x3.ap(), x3[:, :, 0:32])

    nc.compile()
    return nc


def _make_inputs(plan, inputs):
    NSLOT = plan.NSLOT
    xf = inputs["accessibility_features"].astype(np.float32)
    cf = inputs["context_features"].astype(np.float32)

    def col(a):
        return np.ascontiguousarray(np.asarray(a, np.float32).reshape(-1, 1))

    def rep(a, shape):
        return np.ascontiguousarray(
            np.broadcast_to(np.asarray(a, np.float32), shape))

    common = {
        "cg_w1": np.asarray(inputs["cg_w1"], np.float32),
        "cg_b1": col(inputs["cg_b1"]),
        "cg_w2": np.asarray(inputs["cg_w2"], np.float32),
        "cg_b2": col(inputs["cg_b2"]),
        "cg_aw": np.asarray(inputs["cg_aw"], np.float32),
        "cg_ab": col(inputs["cg_ab"]),
        "base_imp": col(inputs["base_imp"]),
        "ln_g": col(inputs["ln_g"]), "ln_b": col(inputs["ln_b"]),
        "fe_w1": np.asarray(inputs["fe_w1"], np.float32),
        "fe_b1": col(inputs["fe_b1"]),
        "fe_w2": np.asarray(inputs["fe_w2"], np.float32),
        "fe_b2": col(inputs["fe_b2"]),
        "gcn1_w": np.asarray(inputs["gcn1_w"], np.float32),
        "gat_w": np.asarray(inputs["gat_w"], np.float32),
        "gcn2_w": np.asarray(inputs["gcn2_w"], np.float32),
        "sp_w1": np.asarray(inputs["sp_w1"], np.float32),
        "sp_b1": col(inputs["sp_b1"]),
        "sp_w2": np.asarray(inputs["sp_w2"], np.float32),
        "sp_b2": col(inputs["sp_b2"]),
        "gcn1_b_r": rep(inputs["gcn1_b"][None, None, :], (128, 1, 64)),
        "gat_b_r": rep(inputs["gat_b"][None, None, :], (128, 1, 64)),
        "gcn2_b_r": rep(inputs["gcn2_b"][None, None, :], (128, 1, 32)),
    }
    asrc = np.asarray(inputs["gat_asrc"], np.float32)
    ast = np.zeros((64, 2), np.float32)
    ast[0:32, 0] = asrc[0]
    ast[32:64, 1] = asrc[1]
    common["asrc_st"] = ast
    adst = np.asarray(inputs["gat_adst"], np.float32)
    adt2 = np.zeros((64, 2), np.float32)
    adt2[0:32, 0] = adst[0]
    adt2[32:64, 1] = adst[1]
    common["adst_st"] = adt2
    for i in (1, 2, 3):
        F = 32 if i == 3 else 64
        for nm in "gbmv":
            common[f"bn{i}_{nm}_r"] = rep(
                np.asarray(inputs[f"bn{i}_{nm}"])[None, None, :], (128, 1, F))

    maps = []
    for c in range(CORES):
        o = plan.orders[c]
        m = o >= 0
        xs = np.zeros((NSLOT, 32), np.float32)
        cs = np.zeros((NSLOT, 5), np.float32)
        xs[m] = xf[o[m]]
        cs[m] = cf[o[m]]
        im = dict(common)
        im["xin"] = np.ascontiguousarray(xs.T)
        im["cin"] = np.ascontiguousarray(cs.T)
        im["degrow"] = plan.deg_row[c]
        im["degpg"] = plan.deg_pg[c]
        im["idxw"] = plan.idxw[c]
        im["maskq"] = plan.maskq[c]
        im["valid"] = plan.valid[c]
        maps.append(im)
    return maps


def run_sim(plan, nc, in_maps):
    """CoreSim single-core run (core 0) — collectives emulated by writing all
    cores' table slices is not possible; instead run with CORES small."""
    raise NotImplementedError


def kernel(**inputs):
    from concourse.bass_utils import run_bass_kernel_spmd

    edge_index = np.asarray(inputs["edge_index"])
    N = inputs["accessibility_features"].shape[0]
    plan = build_plan(edge_index, N)
    nc = build_kernel(plan)
    in_maps = _make_inputs(plan, inputs)

    trace = os.environ.get("KERNEL_TRACE", "0") == "1"
    res = run_bass_kernel_spmd(nc, in_maps, core_ids=list(range(CORES)),
                               trace=trace)
    kernel.last_result = res

    svi = np.zeros(N, np.float32)
    for c in range(CORES):
        o = plan.orders[c]
        m = o >= 0
        flat = res.results[c]["svi"].reshape(plan.NG * 128)
        svi[o[m]] = flat[np.nonzero(m)[0]]
    return svi



# revision 20
# speedup vs baseline: 1.0385x; 1.0184x over previous
"""AccessibilitySVIGNN Trainium2 kernel (8-core SPMD), v2.

See bottom of file for entry point `kernel(**inputs)`.

Design (v2):
- Nodes dst-sharded across 8 cores; per-core slot grid (nodes degree-sorted
  into 128-partition groups, uniform per-group column count across cores so
  one SPMD program works for all cores). Every non-self edge = 1 slot;
  self-loop contributions are added per-group from locally kept h tiles
  (saves ~6% of gather descriptors, the dominant cost).
- Gather tables are bf16, 4 nodes per row so an int16 pack index covers 100k
  nodes; a host-built one-hot bf16 mask selects the quadrant on DVE.
- Gather windows are 16 columns (2048 idx / dma_gather) to amortize the
  per-instruction SWDGE overhead; queue_num rotates 0..3.
- GCN normalization via pre-scaled tables (rsqrt deg on host) + post-scale.
- GAT via gathered [h | alpha_src] rows + online plain-exp softmax.
- Frontend processes 256-wide chunks with float32r matmuls (1 cyc/row).
- Per-group dense compute (gat/gcn2/head) in bf16 on the tensor engine.
"""

import math
import os

import numpy as np

EPS = 1e-5
CORES = 8
WIN = 8  # gather window
CW = 256  # frontend chunk width


class Plan:
    pass


def build_plan(edge_index, n_nodes):
    p = Plan()
    N = n_nodes
    src = edge_index[0].astype(np.int64)
    dst = edge_index[1].astype(np.int64)
    SH = N // CORES
    NG = math.ceil(SH / 128)
    NSLOT = NG * 128
    p.N, p.SH, p.NG, p.NSLOT = N, SH, NG, NSLOT

    cnt = np.bincount(dst, minlength=N)  # real in-edges only (no self loop)
    deg = cnt + 1  # reference degree includes the self loop

    orders = np.full((CORES, NSLOT), -1, np.int64)
    Ks = np.zeros((CORES, NG), np.int64)
    for c in range(CORES):
        nodes = np.arange(c * SH, (c + 1) * SH)
        o = np.argsort(-cnt[nodes], kind="stable")
        orders[c, :SH] = nodes[o]
        cnt_sorted = np.zeros(NSLOT, np.int64)
        cnt_sorted[:SH] = cnt[nodes][o]
        Ks[c] = cnt_sorted.reshape(NG, 128).max(1)
    Kg = np.maximum(Ks.max(0), 1)
    offs = np.concatenate([[0], np.cumsum(Kg)]).astype(np.int64)
    C = int(offs[-1])
    Cpad = ((C + WIN - 1) // WIN) * WIN
    p.Kg, p.offs, p.C, p.Cpad = Kg, offs, C, Cpad
    p.orders = orders

    gslot = np.full(N, -1, np.int64)
    for c in range(CORES):
        m = orders[c] >= 0
        gslot[orders[c][m]] = c * NSLOT + np.nonzero(m)[0]
    p.gslot = gslot

    oe = np.argsort(dst, kind="stable")
    se, sd = src[oe], dst[oe]
    starts = np.searchsorted(sd, np.arange(N))
    j_in_node = np.arange(len(sd)) - starts[sd]

    rank_of = np.full(N, -1, np.int64)
    for c in range(CORES):
        m = orders[c] >= 0
        rank_of[orders[c][m]] = np.nonzero(m)[0]

    idx_grid = np.zeros((CORES, 128, Cpad), np.int16)
    maskq = np.zeros((CORES, 128, Cpad, 4), np.float32)
    inv_row = np.ones((CORES, 1, NSLOT), np.float32)  # rsqrt(deg) per slot
    inv_pg = np.ones((CORES, 128, NG), np.float32)  # rsqrt(deg) per (p, g)

    g_r = np.arange(NSLOT) // 128
    p_r = np.arange(NSLOT) % 128

    for c in range(CORES):
        em = (sd >= c * SH) & (sd < (c + 1) * SH)
        r_e = rank_of[sd[em]]
        cols = offs[g_r[r_e]] + j_in_node[em]
        parts = p_r[r_e]
        gs = gslot[se[em]]
        idx_grid[c, parts, cols] = (gs >> 2).astype(np.int16)
        maskq[c, parts, cols, gs & 3] = 1.0
        m = orders[c] >= 0
        r = np.nonzero(m)[0]
        node = orders[c][m]
        inv_row[c, 0, r] = 1.0 / np.sqrt(deg[node])
        inv_pg[c, p_r[r], g_r[r]] = 1.0 / np.sqrt(deg[node])

    idxw = np.zeros((CORES, 128, Cpad * 8), np.int16)
    for c in range(CORES):
        F = idx_grid[c].T.reshape(-1)
        W16 = F.reshape(-1, 16).T
        for b in range(8):
            idxw[c, b * 16:(b + 1) * 16] = W16

    p.idxw = idxw
    p.valid = (maskq.sum(3, keepdims=True) > 0).astype(np.float32)
    p.maskq = maskq.reshape(CORES, 128, Cpad, 4, 1)
    p.inv_row, p.inv_pg = inv_row, inv_pg.reshape(CORES, 128, NG, 1)

    wins = []
    for w0 in range(0, Cpad, WIN):
        ov = []
        for g in range(NG):
            a, b = max(offs[g], w0), min(offs[g + 1], w0 + WIN)
            if a < b:
                ov.append((g, int(a - w0), int(b - w0)))
        wins.append(ov)
    p.wins = wins
    return p


def build_kernel(plan):
    import concourse.bacc as bacc
    import concourse.mybir as mybir
    import concourse.tile as tile
    from concourse.masks import make_identity

    f32 = mybir.dt.float32
    f32r = mybir.dt.float32r
    bf16 = mybir.dt.bfloat16
    i16 = mybir.dt.int16
    Alu = mybir.AluOpType
    Act = mybir.ActivationFunctionType
    AX = mybir.AxisListType

    NSLOT, NG, Cpad = plan.NSLOT, plan.NG, plan.Cpad
    GS = CORES * NSLOT
    PACKS = GS // 4
    CWk = min(CW, NSLOT)  # frontend chunk width (mini graphs are smaller)
    NCH = NSLOT // CWk

    nc = bacc.Bacc("TRN2", target_bir_lowering=False, debug=False,
                   num_devices=CORES, num_swdge_queues=4)

    def din(name, shape, dt=f32):
        return nc.dram_tensor(name, shape, dt, kind="ExternalInput")

    xin = din("xin", [32, NSLOT])
    cin = din("cin", [5, NSLOT])
    invrow = din("invrow", [1, NSLOT])          # rsqrt(deg) per slot
    invpg_d = din("invpg", [128, NG, 1])        # rsqrt(deg) per (p, g)
    idxw_d = din("idxw", [128, Cpad * 8], i16)
    mask_d = din("maskq", [128, Cpad, 4, 1], bf16)
    valid_d = din("valid", [128, Cpad, 1], bf16)
    wnames = {
        "cg_w1": [5, 32], "cg_b1": [32, 1], "cg_w2": [32, 32], "cg_b2": [32, 1],
        "cg_aw": [32, 32], "cg_ab": [32, 1], "base_imp": [32, 1],
        "ln_g": [32, 1], "ln_b": [32, 1],
        "fe_w1": [32, 64], "fe_b1": [64, 1], "fe_w2": [64, 64], "fe_b2": [64, 1],
        "gcn1_w": [64, 64], "gat_w": [64, 64], "asrc_st": [64, 2],
        "adst_st": [64, 2], "gcn2_w": [64, 32], "sp_w1": [32, 16],
        "sp_w2": [16, 1],
        "sp_b1": [16, 1], "sp_b2": [1, 1],
        "gcn1_b_r": [128, 1, 64], "gat_b_r": [128, 1, 64],
        "gcn2_b_r": [128, 1, 32],
    }
    for i, F in ((1, 64), (2, 64), (3, 32)):
        for nm in "gbmv":
            wnames[f"bn{i}_{nm}_r"] = [128, 1, F]
    wd = {k: din(k, s) for k, s in wnames.items()}

    svi_out = nc.dram_tensor("svi", [NG, 128], f32, kind="ExternalOutput")
    dbg = os.environ.get("KERNEL_DEBUG", "0") == "1"
    if dbg:
        dbg_x1 = nc.dram_tensor("dbg_x1", [128, NG, 64], bf16, kind="ExternalOutput")
        dbg_x2 = nc.dram_tensor("dbg_x2", [128, NG, 64], bf16, kind="ExternalOutput")
        dbg_x3 = nc.dram_tensor("dbg_x3", [128, NG, 32], bf16, kind="ExternalOutput")

    cc1_in = nc.dram_tensor("cc1_in", [NSLOT // 4, 128], f32, kind="Internal")
    table1 = nc.dram_tensor("table1", [PACKS, 128], f32, kind="Internal",
                            addr_space="Shared")
    cc2_in = nc.dram_tensor("cc2_in", [NSLOT // 4, 192], f32, kind="Internal")
    table2 = nc.dram_tensor("table2", [PACKS, 192], f32, kind="Internal",
                            addr_space="Shared")
    cc3_in = nc.dram_tensor("cc3_in", [NSLOT // 4, 64], f32, kind="Internal")
    table3 = nc.dram_tensor("table3", [PACKS, 64], f32, kind="Internal",
                            addr_space="Shared")

    RG = [list(range(CORES))]

    with tile.TileContext(nc) as tc:
        with (
            tc.tile_pool(name="resident", bufs=1) as rp,
            tc.tile_pool(name="work", bufs=2) as wp,
            tc.tile_pool(name="psum", bufs=2, space="PSUM") as pp,
            tc.tile_pool(name="psumT", bufs=1, space="PSUM") as ppt,
        ):
            consts = rp.tile([128, 1920], f32, tag="consts", name="consts")
            _cur = [0]

            def calloc(P, W):
                c0 = _cur[0]
                _cur[0] += W
                assert _cur[0] <= 1920
                return consts[0:P, c0:c0 + W]

            def cload(name):
                sh = wd[name].shape
                P = sh[0]
                Wn = int(np.prod(sh[1:]))
                sl = calloc(P, Wn)
                nc.sync.dma_start(sl, wd[name].ap().rearrange(
                    {2: "a b -> a b", 3: "a b c -> a (b c)",
                     4: "a b c d -> a (b c d)"}[len(sh)]))
                view = sl
                if len(sh) == 3:
                    view = sl.rearrange("a (b c) -> a b c", b=sh[1])
                elif len(sh) == 4:
                    view = sl.rearrange("a (b c d) -> a b c d", b=sh[1], c=sh[2])
                return view

            W = {k: cload(k) for k in wnames}

            ident = calloc(128, 128)
            make_identity(nc, ident)
            ones_col = calloc(128, 1)
            nc.vector.memset(ones_col, 1.0)
            ones_row64 = calloc(1, 64)
            nc.vector.memset(ones_row64, 1.0)

            def bn_scale_shift(i, F, bias_r):
                """Returns sc, sh2 with gcn/gat bias folded into the shift."""
                sc = calloc(128, F).rearrange("a (b c) -> a b c", b=1)
                sh = calloc(128, F).rearrange("a (b c) -> a b c", b=1)
                t = calloc(128, F).rearrange("a (b c) -> a b c", b=1)
                nc.vector.tensor_scalar(t, W[f"bn{i}_v_r"], EPS, None, Alu.add)
                nc.vector.reciprocal(t, t)
                nc.scalar.activation(t, t, Act.Sqrt)
                nc.vector.tensor_tensor(out=sc, in0=W[f"bn{i}_g_r"], in1=t,
                                        op=Alu.mult)
                # sh2 = b - m*sc + bias*sc = b + (bias - m)*sc
                nc.vector.tensor_tensor(out=t, in0=bias_r, in1=W[f"bn{i}_m_r"],
                                        op=Alu.subtract)
                nc.vector.tensor_tensor(out=t, in0=t, in1=sc, op=Alu.mult)
                nc.vector.tensor_tensor(out=sh, in0=W[f"bn{i}_b_r"], in1=t,
                                        op=Alu.add)
                return sc, sh

            inv_pg = rp.tile([128, NG, 1], f32, tag="invpg", name="invpg")
            nc.sync.dma_start(inv_pg[:], invpg_d.ap())

            s_mask = rp.tile([128, Cpad, 4, 1], bf16, tag="mask", name="mask")
            nc.sync.dma_start(s_mask[:], mask_d.ap())
            s_valid = rp.tile([128, Cpad, 1], bf16, tag="valid", name="valid")
            nc.sync.dma_start(s_valid[:], valid_d.ap())

            def mm(lhsT_ap, rhs_ap, m, w, tag="mmps", fast=False):
                ps = pp.tile([m, w], f32, tag=tag, name=tag)
                nc.tensor.matmul(ps[:], lhsT_ap, rhs_ap, start=True,
                                 stop=True)
                return ps

            def bcast_row(row_ap, F, w, fast=False):
                return mm(ones_row64[:, :F], row_ap, F, w, tag="bcps",
                          fast=fast)

            def psum_sum32(rhs_ap, w, fast=False):
                return mm(ones_col[0:32, :], rhs_ap, 1, w, tag="s32ps",
                          fast=fast)

            def transpose_to(in_ap, k, m, tag="tpps"):
                ps = ppt.tile([m, k], f32, tag=tag, name=tag)
                nc.tensor.transpose(ps[:], in_ap, ident[0:k, 0:k])
                return ps

            # group g's acc is complete once the window covering column
            # offs[g+1]-1 has been accumulated; emit per-group epilogues
            # (bn+relu, next table build, head) right there so vector/PE
            # work overlaps the gpsimd-bound gather descgen of later windows
            done_after = {}
            for g in range(NG):
                wi = (int(plan.offs[g + 1]) - 1) // WIN
                done_after.setdefault(wi, []).append(g)

            # frontend scratch: 4 overlaid data slots (lifetimes disjoint)
            # plus 7 single-row stat slots, all 256 wide at base partition 0
            FOFF = dict(A=0, B=256, C=512, D=768,
                        st0=1024, st1=1280, st2=1536, st3=1792, st4=2048,
                        st5=2304, st6=2560)

            def scf_tile():
                return wp.tile([128, 2816], f32, tag="scfF", name="scfF")

            # per-group scratch (128-wide slots)
            GOFF = dict(xt=0, stk=128, h2=256, adc=384, st=512, nm=640)

            def scg_tile():
                return wp.tile([128, 768], f32, tag="scfG", name="scfG")

            # ---------------- frontend + table1 ----------------
            x1 = rp.tile([128, NG, 64], f32, tag="bigB", name="x1")
            h1_own = rp.tile([128, NG, 64], bf16, tag="ownA", name="h1_own")

            for ci in range(NCH):
                c0 = ci * CWk
                w = CWk
                scf = scf_tile()

                def S(nm, P, ww=None):
                    return scf[0:P, FOFF[nm]:FOFF[nm] + (ww or w)]

                def ST(i):
                    return scf[0:1, FOFF[f"st{i}"]:FOFF[f"st{i}"] + w]

                xc = S("A", 32)
                nc.sync.dma_start(xc, xin.ap()[:, c0:c0 + w])
                ctc = S("B", 5)
                nc.sync.dma_start(ctc, cin.ap()[:, c0:c0 + w])
                invc = ST(6)
                nc.sync.dma_start(invc, invrow.ap()[:, c0:c0 + w])

                ps = mm(W["cg_w1"], ctc, 32, w, fast=True)
                ce1 = S("C", 32)
                nc.scalar.activation(ce1, ps[:], Act.Relu, bias=W["cg_b1"])
                ps = mm(W["cg_w2"], ce1, 32, w, fast=True)
                ce2 = S("D", 32)
                nc.vector.tensor_scalar(ce2, ps[:], W["cg_b2"], None, Alu.add)
                ps = mm(W["cg_aw"], ce2, 32, w, fast=True)
                ez = S("C", 32)  # ce1 dead
                nc.scalar.activation(ez, ps[:], Act.Exp, bias=W["cg_ab"])
                psS = psum_sum32(ez, w, fast=True)
                rS = ST(0)
                nc.vector.reciprocal(rS, psS[:])
                psb = bcast_row(rS, 32, w, fast=True)
                att = S("B", 32)  # ctc dead
                nc.vector.tensor_tensor(out=att, in0=ez, in1=psb[:], op=Alu.mult)
                xg = S("C", 32)  # ez dead
                nc.vector.scalar_tensor_tensor(out=xg, in0=att,
                                               scalar=W["base_imp"], in1=xc,
                                               op0=Alu.mult, op1=Alu.mult)
                ps1 = psum_sum32(xg, w, fast=True)
                sq = S("B", 32)  # att dead
                nc.vector.tensor_tensor(out=sq, in0=xg, in1=xg, op=Alu.mult)
                ps2 = psum_sum32(sq, w, fast=True)
                mu = ST(1)
                nc.vector.tensor_scalar(mu, ps1[:], 1.0 / 32, None, Alu.mult)
                var = ST(2)
                nc.vector.tensor_scalar(var, ps2[:], 1.0 / 32, None, Alu.mult)
                musq = ST(3)
                nc.vector.tensor_tensor(out=musq, in0=mu, in1=mu, op=Alu.mult)
                nc.vector.tensor_tensor(out=var, in0=var, in1=musq,
                                        op=Alu.subtract)
                nc.vector.tensor_scalar(var, var, EPS, None, Alu.add)
                nc.vector.reciprocal(var, var)
                rstd = ST(4)
                nc.scalar.activation(rstd, var, Act.Sqrt)
                mrs = ST(5)
                nc.vector.tensor_tensor(out=mrs, in0=mu, in1=rstd, op=Alu.mult)
                psA = bcast_row(rstd, 32, w, fast=True)
                xn = S("A", 32)  # xc dead
                nc.vector.tensor_tensor(out=xn, in0=xg, in1=psA[:], op=Alu.mult)
                psB = bcast_row(mrs, 32, w, fast=True)
                nc.vector.tensor_tensor(out=xn, in0=xn, in1=psB[:],
                                        op=Alu.subtract)
                nc.vector.tensor_scalar(xn, xn, W["ln_g"], W["ln_b"],
                                        Alu.mult, Alu.add)
                ps = mm(W["fe_w1"], xn, 64, w, fast=True)
                f1 = S("B", 64)  # sq dead
                nc.scalar.activation(f1, ps[:], Act.Relu, bias=W["fe_b1"])
                ps = mm(W["fe_w2"], f1, 64, w, fast=True)
                x0 = S("C", 64)  # xg dead
                nc.scalar.activation(x0, ps[:], Act.Relu, bias=W["fe_b2"])
                ps = mm(W["gcn1_w"], x0, 64, w, fast=True)
                psI = bcast_row(invc, 64, w, fast=True)
                h1 = S("D", 64)  # ce2 dead
                nc.vector.tensor_copy(out=h1, in_=ps[:])
                nc.vector.tensor_tensor(out=h1, in0=h1, in1=psI[:],
                                        op=Alu.mult)
                # keep own (pre-scaled) h1 rows for the self-loop term
                for half in range(CWk // 128):
                    tp = transpose_to(h1[:, half * 128:(half + 1) * 128],
                                      64, 128)
                    nc.vector.tensor_copy(
                        out=h1_own[:, (CWk // 128) * ci + half], in_=tp[:])
                # table1 chunk: 4-node bf16 pack rows
                pk = wp.tile([CWk // 4, 256], bf16, tag="pk1", name="pk1")
                for q in range(4):
                    tp = transpose_to(h1[:, q::4], 64, CWk // 4)
                    nc.vector.tensor_copy(out=pk[:, q * 64:q * 64 + 64],
                                          in_=tp[:])
                nc.sync.dma_start(
                    cc1_in.ap().bitcast(bf16)[c0 // 4:c0 // 4 + CWk // 4, :],
                    pk[:])

            nc.gpsimd.collective_compute(
                "AllGather", Alu.bypass, replica_groups=RG,
                ins=[cc1_in.ap()], outs=[table1.ap()])

            # ---------------- generic GCN conv ----------------
            def gcn_conv(table, F, acc, after_group=None):
                nc.vector.memset(acc, 0.0)
                for wi, ov in enumerate(plan.wins):
                    c0 = wi * WIN
                    idxt = wp.tile([128, WIN * 8], i16, tag="idxt", name="idxt")
                    nc.sync.dma_start(idxt[:],
                                      idxw_d.ap()[:, c0 * 8:(c0 + WIN) * 8])
                    gt = wp.tile([128, WIN * 2 * F], f32, tag="gt", name="gt")
                    nc.gpsimd.dma_gather(
                        gt[:].rearrange("p (w e) -> p w e", e=2 * F),
                        table.ap(), idxt[:], 128 * WIN, 128 * WIN, 2 * F,
                        queue_num=wi % 4)
                    mq = wp.tile([128, WIN, 4, F], bf16, tag="mq", name="mq")
                    nc.vector.tensor_tensor(
                        out=mq[:],
                        in0=gt[:].bitcast(bf16).rearrange(
                            "p (w q f) -> p w q f", w=WIN, q=4),
                        in1=s_mask[:, c0:c0 + WIN].to_broadcast(
                            [128, WIN, 4, F]),
                        op=Alu.mult)
                    scw = wp.tile([128, 512], f32, tag="scw", name="scw")
                    for oi, (g, a, b) in enumerate(ov):
                        red = scw[:, 322 + 64 * (oi % 2):322 + 64 * (oi % 2) + F]
                        nc.vector.tensor_reduce(
                            out=red,
                            in_=mq[:, a:b].rearrange("p c q f -> p f c q"),
                            axis=AX.XY, op=Alu.add)
                        nc.vector.tensor_tensor(out=acc[:, g], in0=acc[:, g],
                                                in1=red, op=Alu.add)
                    if after_group is not None:
                        for g in done_after.get(wi, []):
                            after_group(g)

            def post_gcn_group(acc, x, g, F, h_own, sc, sh2):
                scw = wp.tile([128, 512], f32, tag="scw", name="scw")
                t = scw[:, 0:F].rearrange("p (c f) -> p c f", c=1)
                # self loop: acc + own prescaled h, then * rsqrt(deg_dst)
                nc.vector.tensor_tensor(
                    out=t, in0=acc[:, g:g + 1, 0:F],
                    in1=h_own[:, g:g + 1, 0:F], op=Alu.add)
                xg = x[:, g:g + 1, 0:F]
                nc.vector.scalar_tensor_tensor(
                    out=t, in0=t, scalar=inv_pg[:, g],
                    in1=sc.to_broadcast([128, 1, F]),
                    op0=Alu.mult, op1=Alu.mult)
                nc.vector.tensor_tensor(
                    out=xg, in0=t, in1=sh2.to_broadcast([128, 1, F]),
                    op=Alu.add)
                nc.scalar.activation(xg, xg, Act.Relu)

            # ---------------- conv1 + interleaved table2 build ----------
            ad_pg = rp.tile([128, NG, 2], f32, tag="adpg", name="adpg")
            sc1, sh1 = bn_scale_shift(1, 64, W["gcn1_b_r"])
            h2_own = rp.tile([128, NG, 66], bf16, tag="ownB", name="h2_own")

            def fe2_chunk(g):
                scb = scg_tile()
                xt = scb[0:64, GOFF["xt"]:GOFF["xt"] + 128]
                tp = transpose_to(x1[:, g], 128, 64)
                nc.vector.tensor_copy(out=xt, in_=tp[:])
                psh = mm(W["gat_w"], xt, 64, 128)
                stk = scb[0:66, GOFF["stk"]:GOFF["stk"] + 128]
                nc.vector.tensor_copy(out=stk[0:64], in_=psh[:])
                psa = mm(W["asrc_st"], stk[0:64], 2, 128)
                nc.vector.tensor_copy(out=stk[64:66], in_=psa[:])
                psd = mm(W["adst_st"], stk[0:64], 2, 128)
                adc = scb[0:2, GOFF["adc"]:GOFF["adc"] + 128]
                nc.vector.tensor_copy(out=adc, in_=psd[:])
                tpd = transpose_to(adc, 2, 128)
                nc.vector.tensor_copy(out=ad_pg[:, g], in_=tpd[:])
                tph = transpose_to(stk, 66, 128)
                nc.vector.tensor_copy(out=h2_own[:, g], in_=tph[:])
                # table2 chunk: 4-node bf16 pack rows, 96 cols per node
                pk = wp.tile([32, 384], bf16, tag="pk2", name="pk2")
                nc.vector.memset(pk[:], 0.0)
                for q in range(4):
                    tp2 = transpose_to(stk[:, q::4], 66, 32)
                    nc.vector.tensor_copy(out=pk[:, q * 96:q * 96 + 66],
                                          in_=tp2[:])
                nc.sync.dma_start(cc2_in.ap().bitcast(bf16)[g * 32:(g + 1) * 32, :],
                                  pk[:])

            acc1 = rp.tile([128, NG, 64], f32, tag="bigA", name="acc1")

            def post1_and_fe2(g):
                post_gcn_group(acc1, x1, g, 64, h1_own, sc1, sh1)
                fe2_chunk(g)

            gcn_conv(table1, 64, acc1[:], after_group=post1_and_fe2)
            if dbg:
                nc.sync.dma_start(dbg_x1.ap(), x1[:])

            # expand ad_pg to per-column layout (overlaps AllGather)
            ad_col = rp.tile([128, Cpad, 2], bf16, tag="adcol", name="adcol")
            if plan.C < Cpad:
                nc.vector.memset(ad_col[:, plan.C:Cpad], 0.0)
            for g in range(NG):
                a, b = int(plan.offs[g]), int(plan.offs[g + 1])
                nc.vector.tensor_copy(
                    out=ad_col[:, a:b],
                    in_=ad_pg[:, g:g + 1].to_broadcast([128, b - a, 2]))

            nc.gpsimd.collective_compute(
                "AllGather", Alu.bypass, replica_groups=RG,
                ins=[cc2_in.ap()], outs=[table2.ap()])

            # ---------------- GAT conv + interleaved table3 build --------
            N_pg = rp.tile([128, NG, 64], f32, tag="bigA", name="N_pg")
            nc.vector.memset(N_pg[:], 0.0)
            S_pg = rp.tile([128, NG, 2], f32, tag="spg", name="S_pg")
            nc.vector.memset(S_pg[:], 0.0)
            sc2, sh2g = bn_scale_shift(2, 64, W["gat_b_r"])
            rS2 = rp.tile([128, NG, 2], f32, tag="rs2", name="rS2")
            x2 = rp.tile([128, NG, 64], f32, tag="bigB", name="x2")
            h3_own = rp.tile([128, NG, 64], bf16, tag="ownA", name="h3_own")

            def fe3_chunk(g):
                scb = scg_tile()
                scg = scb
                xt = scb[0:64, GOFF["xt"]:GOFF["xt"] + 128]
                tp = transpose_to(x2[:, g], 128, 64)
                nc.vector.tensor_copy(out=xt, in_=tp[:])
                ps = mm(W["gcn2_w"], xt, 32, 128)
                invc = scg[0:1, GOFF["st"]:GOFF["st"] + 128]
                nc.sync.dma_start(invc, invrow.ap()[:, g * 128:(g + 1) * 128])
                psI = bcast_row(invc, 32, 128)
                h3 = scb[0:32, GOFF["h2"]:GOFF["h2"] + 128]
                nc.vector.tensor_copy(out=h3, in_=ps[:])
                nc.vector.tensor_tensor(out=h3, in0=h3, in1=psI[:],
                                        op=Alu.mult)
                tph = transpose_to(h3, 32, 128)
                nc.vector.tensor_copy(out=h3_own[:, g, 0:32], in_=tph[:])
                pk = wp.tile([32, 128], bf16, tag="pk3", name="pk3")
                for q in range(4):
                    tp2 = transpose_to(h3[:, q::4], 32, 32)
                    nc.vector.tensor_copy(out=pk[:, q * 32:q * 32 + 32],
                                          in_=tp2[:])
                nc.sync.dma_start(cc3_in.ap().bitcast(bf16)[g * 32:(g + 1) * 32, :],
                                  pk[:])

            def post2_and_fe3(g):
                scw = wp.tile([128, 512], f32, tag="scw", name="scw")
                # self-loop alpha term
                es = scw[:, 0:2].rearrange("p (c h) -> p c h", c=1)
                nc.vector.tensor_tensor(
                    out=es, in0=h2_own[:, g:g + 1, 64:66],
                    in1=ad_pg[:, g:g + 1], op=Alu.add)
                es2 = scw[:, 2:4].rearrange("p (c h) -> p c h", c=1)
                nc.vector.tensor_scalar(es2, es, 0.2, None, Alu.mult)
                nc.vector.tensor_tensor(out=es, in0=es, in1=es2, op=Alu.max)
                nc.scalar.activation(es, es, Act.Exp)
                nc.vector.tensor_tensor(out=S_pg[:, g], in0=S_pg[:, g],
                                        in1=es[:, 0], op=Alu.add)
                for h in range(2):
                    nc.vector.scalar_tensor_tensor(
                        out=N_pg[:, g:g + 1, 32 * h:32 * h + 32],
                        in0=h2_own[:, g:g + 1, 32 * h:32 * h + 32],
                        scalar=es[:, 0, h:h + 1],
                        in1=N_pg[:, g:g + 1, 32 * h:32 * h + 32],
                        op0=Alu.mult, op1=Alu.add)
                rg = rS2[:, g:g + 1]
                nc.vector.tensor_scalar(rg, S_pg[:, g:g + 1], 1e-16, None,
                                        Alu.add)
                nc.vector.reciprocal(rg, rg)
                xg = x2[:, g:g + 1]
                for h in range(2):
                    nc.vector.scalar_tensor_tensor(
                        out=xg[:, :, 32 * h:32 * h + 32],
                        in0=N_pg[:, g:g + 1, 32 * h:32 * h + 32],
                        scalar=rg[:, :, h],
                        in1=sc2[:, :, 32 * h:32 * h + 32].to_broadcast(
                            [128, 1, 32]),
                        op0=Alu.mult, op1=Alu.mult)
                nc.vector.tensor_tensor(
                    out=xg, in0=xg, in1=sh2g.to_broadcast([128, 1, 64]),
                    op=Alu.add)
                nc.scalar.activation(xg, xg, Act.Relu)
                fe3_chunk(g)

            for wi, ov in enumerate(plan.wins):
                c0 = wi * WIN
                idxt = wp.tile([128, WIN * 8], i16, tag="idxt", name="idxt")
                nc.sync.dma_start(idxt[:], idxw_d.ap()[:, c0 * 8:(c0 + WIN) * 8])
                gt = wp.tile([128, WIN * 192], f32, tag="gt", name="gt")
                nc.gpsimd.dma_gather(
                    gt[:].rearrange("p (w e) -> p w e", e=192), table2.ap(),
                    idxt[:], 128 * WIN, 128 * WIN, 192, queue_num=wi % 4)
                mq = wp.tile([128, WIN, 4, 66], bf16, tag="mq", name="mq")
                gtb = gt[:].bitcast(bf16).rearrange(
                    "p (w q f) -> p w q f", w=WIN, q=4)
                nc.vector.tensor_tensor(
                    out=mq[:], in0=gtb[:, :, :, 0:66],
                    in1=s_mask[:, c0:c0 + WIN].to_broadcast([128, WIN, 4, 66]),
                    op=Alu.mult)
                sel = wp.tile([128, WIN, 66], f32, tag="sel", name="sel")
                nc.vector.tensor_reduce(
                    out=sel[:], in_=mq[:].rearrange("p c q f -> p c f q"),
                    axis=AX.X, op=Alu.add)
                scw = wp.tile([128, 512], f32, tag="scw", name="scw")
                # whole-window attention math (no per-ov splitting)
                e = scw[:, 0:2 * WIN].rearrange("p (c h) -> p c h", h=2)
                nc.vector.tensor_tensor(
                    out=e, in0=sel[:, :, 64:66],
                    in1=ad_col[:, c0:c0 + WIN], op=Alu.add)
                e2 = scw[:, 2 * WIN:4 * WIN].rearrange("p (c h) -> p c h", h=2)
                nc.vector.tensor_scalar(e2, e, 0.2, None, Alu.mult)
                nc.vector.tensor_tensor(out=e, in0=e, in1=e2, op=Alu.max)
                nc.scalar.activation(e, e, Act.Exp)
                nc.vector.tensor_tensor(
                    out=e, in0=e,
                    in1=s_valid[:, c0:c0 + WIN].to_broadcast([128, WIN, 2]),
                    op=Alu.mult)
                nmt = wp.tile([128, WIN, 2, 32], bf16, tag="nmt", name="nmt")
                e4 = scw[:, 0:2 * WIN].rearrange("p (c h f) -> p c h f",
                                                 h=2, f=1)
                nc.vector.tensor_tensor(
                    out=nmt[:],
                    in0=sel[:, :, 0:64].rearrange("p c (h f) -> p c h f", h=2),
                    in1=e4.to_broadcast([128, WIN, 2, 32]),
                    op=Alu.mult)
                red0 = 4 * WIN
                for (g, a, b) in ov:
                    red2 = scw[:, red0:red0 + 2]
                    nc.vector.tensor_reduce(
                        out=red2, in_=e[:, a:b].rearrange("p c h -> p h c"),
                        axis=AX.X, op=Alu.add)
                    nc.vector.tensor_tensor(out=S_pg[:, g], in0=S_pg[:, g],
                                            in1=red2, op=Alu.add)
                    redh = scw[:, red0 + 2:red0 + 66]
                    nc.vector.tensor_reduce(
                        out=redh.rearrange("p (h f) -> p h f", h=2),
                        in_=nmt[:, a:b].rearrange("p c h f -> p h f c"),
                        axis=AX.X, op=Alu.add)
                    nc.vector.tensor_tensor(out=N_pg[:, g], in0=N_pg[:, g],
                                            in1=redh, op=Alu.add)
                for g in done_after.get(wi, []):
                    post2_and_fe3(g)

            if dbg:
                nc.sync.dma_start(dbg_x2.ap(), x2[:])

            nc.gpsimd.collective_compute(
                "AllGather", Alu.bypass, replica_groups=RG,
                ins=[cc3_in.ap()], outs=[table3.ap()])

            # ---------------- conv3 + interleaved head ----------------
            sc3, sh3 = bn_scale_shift(3, 32, W["gcn2_b_r"])
            x3 = rp.tile([128, NG, 64], f32, tag="bigB", name="x3")
            acc3 = rp.tile([128, NG, 64], f32, tag="bigA", name="acc3")

            def head_chunk(g):
                scb = scg_tile()
                scg = scb
                tp = transpose_to(x3[:, g, 0:32], 128, 32)
                xh = scb[0:32, GOFF["xt"]:GOFF["xt"] + 128]
                nc.vector.tensor_copy(out=xh, in_=tp[:])
                ps = mm(W["sp_w1"], xh, 16, 128)
                hh = scb[0:16, GOFF["stk"]:GOFF["stk"] + 128]
                nc.scalar.activation(hh, ps[:], Act.Relu, bias=W["sp_b1"])
                ps = mm(W["sp_w2"], hh, 1, 128)
                sv = scg[0:1, GOFF["adc"]:GOFF["adc"] + 128]
                nc.scalar.activation(sv, ps[:], Act.Sigmoid, bias=W["sp_b2"])
                nc.sync.dma_start(svi_out.ap()[g:g + 1, :], sv)

            def post3_and_head(g):
                post_gcn_group(acc3, x3, g, 32, h3_own, sc3, sh3)
                head_chunk(g)

            gcn_conv(table3, 32, acc3[:, :, 0:32], after_group=post3_and_head)
            if dbg:
                nc.sync.dma_start(dbg_x3.ap(), x3[:, :, 0:32])

    nc.compile()
    return nc


def _make_inputs(plan, inputs):
    import ml_dtypes
    bf16 = ml_dtypes.bfloat16

    NSLOT = plan.NSLOT
    xf = inputs["accessibility_features"].astype(np.float32)
    cf = inputs["context_features"].astype(np.float32)

    def col(a):
        return np.ascontiguousarray(np.asarray(a, np.float32).reshape(-1, 1))

    def rep(a, shape):
        return np.ascontiguousarray(
            np.broadcast_to(np.asarray(a, np.float32), shape))

    common = {
        "cg_w1": np.asarray(inputs["cg_w1"], np.float32),
        "cg_b1": col(inputs["cg_b1"]),
        "cg_w2": np.asarray(inputs["cg_w2"], np.float32),
        "cg_b2": col(inputs["cg_b2"]),
        "cg_aw": np.asarray(inputs["cg_aw"], np.float32),
        "cg_ab": col(inputs["cg_ab"]),
        "base_imp": col(inputs["base_imp"]),
        "ln_g": col(inputs["ln_g"]), "ln_b": col(inputs["ln_b"]),
        "fe_w1": np.asarray(inputs["fe_w1"], np.float32),
        "fe_b1": col(inputs["fe_b1"]),
        "fe_w2": np.asarray(inputs["fe_w2"], np.float32),
        "fe_b2": col(inputs["fe_b2"]),
        "gcn1_w": np.asarray(inputs["gcn1_w"], np.float32),
        "gat_w": np.asarray(inputs["gat_w"], np.float32),
        "gcn2_w": np.asarray(inputs["gcn2_w"], np.float32),
        "sp_w1": np.asarray(inputs["sp_w1"], np.float32),
        "sp_b1": col(inputs["sp_b1"]),
        "sp_w2": np.asarray(inputs["sp_w2"], np.float32),
        "sp_b2": col(inputs["sp_b2"]),
        "gcn1_b_r": rep(inputs["gcn1_b"][None, None, :], (128, 1, 64)),
        "gat_b_r": rep(inputs["gat_b"][None, None, :], (128, 1, 64)),
        "gcn2_b_r": rep(inputs["gcn2_b"][None, None, :], (128, 1, 32)),
    }
    asrc = np.asarray(inputs["gat_asrc"], np.float32)
    ast = np.zeros((64, 2), np.float32)
    ast[0:32, 0] = asrc[0]
    ast[32:64, 1] = asrc[1]
    common["asrc_st"] = ast
    adst = np.asarray(inputs["gat_adst"], np.float32)
    adt2 = np.zeros((64, 2), np.float32)
    adt2[0:32, 0] = adst[0]
    adt2[32:64, 1] = adst[1]
    common["adst_st"] = adt2
    for i in (1, 2, 3):
        F = 32 if i == 3 else 64
        for nm in "gbmv":
            common[f"bn{i}_{nm}_r"] = rep(
                np.asarray(inputs[f"bn{i}_{nm}"])[None, None, :], (128, 1, F))

    maps = []
    for c in range(CORES):
        o = plan.orders[c]
        m = o >= 0
        xs = np.zeros((NSLOT, 32), np.float32)
        cs = np.zeros((NSLOT, 5), np.float32)
        xs[m] = xf[o[m]]
        cs[m] = cf[o[m]]
        im = dict(common)
        im["xin"] = np.ascontiguousarray(xs.T)
        im["cin"] = np.ascontiguousarray(cs.T)
        im["invrow"] = plan.inv_row[c]
        im["invpg"] = plan.inv_pg[c]
        im["idxw"] = plan.idxw[c]
        im["maskq"] = plan.maskq[c].astype(bf16)
        im["valid"] = plan.valid[c].astype(bf16)
        maps.append(im)
    return maps


def kernel(**inputs):
    from concourse.bass_utils import run_bass_kernel_spmd

    edge_index = np.asarray(inputs["edge_index"])
    N = inputs["accessibility_features"].shape[0]
    plan = build_plan(edge_index, N)
    nc = build_kernel(plan)
    in_maps = _make_inputs(plan, inputs)

    trace = os.environ.get("KERNEL_TRACE", "0") == "1"
    res = run_bass_kernel_spmd(nc, in_maps, core_ids=list(range(CORES)),
                               trace=trace)
    kernel.last_result = res

    svi = np.zeros(N, np.float32)
    for c in range(CORES):
        o = plan.orders[c]
        m = o >= 0
        flat = res.results[c]["svi"].reshape(plan.NG * 128)
        svi[o[m]] = flat[np.nonzero(m)[0]]
    return svi


# revision 28
# speedup vs baseline: 1.1500x; 1.1074x over previous
"""AccessibilitySVIGNN Trainium2 kernel (8-core SPMD), v2.

See bottom of file for entry point `kernel(**inputs)`.

Design (v2):
- Nodes dst-sharded across 8 cores; per-core slot grid (nodes degree-sorted
  into 128-partition groups, uniform per-group column count across cores so
  one SPMD program works for all cores). Every non-self edge = 1 slot;
  self-loop contributions are added per-group from locally kept h tiles
  (saves ~6% of gather descriptors, the dominant cost).
- Gather tables are bf16, 4 nodes per row so an int16 pack index covers 100k
  nodes; a host-built one-hot bf16 mask selects the quadrant on DVE.
- Gather windows are 16 columns (2048 idx / dma_gather) to amortize the
  per-instruction SWDGE overhead; queue_num rotates 0..3.
- GCN normalization via pre-scaled tables (rsqrt deg on host) + post-scale.
- GAT via gathered [h | alpha_src] rows + online plain-exp softmax.
- Frontend processes 256-wide chunks with float32r matmuls (1 cyc/row).
- Per-group dense compute (gat/gcn2/head) in bf16 on the tensor engine.
"""

import math
import os

import numpy as np

EPS = 1e-5
CORES = 8
WIN = 8  # gather window: 1024 idxs per dma_gather (2048 overflows the desc ring)
CW = 256  # frontend chunk width


class Plan:
    pass


def build_plan(edge_index, n_nodes):
    p = Plan()
    N = n_nodes
    src = edge_index[0].astype(np.int64)
    dst = edge_index[1].astype(np.int64)
    SH = N // CORES
    NG = math.ceil(SH / 128)
    NSLOT = NG * 128
    p.N, p.SH, p.NG, p.NSLOT = N, SH, NG, NSLOT

    cnt = np.bincount(dst, minlength=N)  # real in-edges only (no self loop)
    deg = cnt + 1  # reference degree includes the self loop

    orders = np.full((CORES, NSLOT), -1, np.int64)
    Ks = np.zeros((CORES, NG), np.int64)
    for c in range(CORES):
        nodes = np.arange(c * SH, (c + 1) * SH)
        o = np.argsort(-cnt[nodes], kind="stable")
        orders[c, :SH] = nodes[o]
        cnt_sorted = np.zeros(NSLOT, np.int64)
        cnt_sorted[:SH] = cnt[nodes][o]
        Ks[c] = cnt_sorted.reshape(NG, 128).max(1)
    Kg = np.maximum(Ks.max(0), 1)
    offs = np.concatenate([[0], np.cumsum(Kg)]).astype(np.int64)
    C = int(offs[-1])
    Cpad = ((C + WIN - 1) // WIN) * WIN
    p.Kg, p.offs, p.C, p.Cpad = Kg, offs, C, Cpad
    p.orders = orders

    gslot = np.full(N, -1, np.int64)
    for c in range(CORES):
        m = orders[c] >= 0
        gslot[orders[c][m]] = c * NSLOT + np.nonzero(m)[0]
    p.gslot = gslot

    oe = np.argsort(dst, kind="stable")
    se, sd = src[oe], dst[oe]
    starts = np.searchsorted(sd, np.arange(N))
    j_in_node = np.arange(len(sd)) - starts[sd]

    rank_of = np.full(N, -1, np.int64)
    for c in range(CORES):
        m = orders[c] >= 0
        rank_of[orders[c][m]] = np.nonzero(m)[0]

    idx_grid = np.zeros((CORES, 128, Cpad), np.int16)
    maskq = np.zeros((CORES, 128, Cpad, 4), np.float32)
    inv_row = np.ones((CORES, 1, NSLOT), np.float32)  # rsqrt(deg) per slot
    inv_pg = np.ones((CORES, 128, NG), np.float32)  # rsqrt(deg) per (p, g)

    g_r = np.arange(NSLOT) // 128
    p_r = np.arange(NSLOT) % 128

    for c in range(CORES):
        em = (sd >= c * SH) & (sd < (c + 1) * SH)
        r_e = rank_of[sd[em]]
        cols = offs[g_r[r_e]] + j_in_node[em]
        parts = p_r[r_e]
        gs = gslot[se[em]]
        idx_grid[c, parts, cols] = (gs >> 2).astype(np.int16)
        maskq[c, parts, cols, gs & 3] = 1.0
        m = orders[c] >= 0
        r = np.nonzero(m)[0]
        node = orders[c][m]
        inv_row[c, 0, r] = 1.0 / np.sqrt(deg[node])
        inv_pg[c, p_r[r], g_r[r]] = 1.0 / np.sqrt(deg[node])

    idxw = np.zeros((CORES, 128, Cpad * 8), np.int16)
    for c in range(CORES):
        F = idx_grid[c].T.reshape(-1)
        W16 = F.reshape(-1, 16).T
        for b in range(8):
            idxw[c, b * 16:(b + 1) * 16] = W16

    p.idxw = idxw
    p.valid = (maskq.sum(3, keepdims=True) > 0).astype(np.float32)
    p.maskq = maskq.reshape(CORES, 128, Cpad, 4, 1)
    p.inv_row, p.inv_pg = inv_row, inv_pg.reshape(CORES, 128, NG, 1)

    wins = []
    for w0 in range(0, Cpad, WIN):
        ov = []
        for g in range(NG):
            a, b = max(offs[g], w0), min(offs[g + 1], w0 + WIN)
            if a < b:
                ov.append((g, int(a - w0), int(b - w0)))
        wins.append(ov)
    p.wins = wins
    return p


def build_kernel(plan):
    import concourse.bacc as bacc
    import concourse.mybir as mybir
    import concourse.tile as tile
    from concourse.masks import make_identity

    f32 = mybir.dt.float32
    f32r = mybir.dt.float32r
    bf16 = mybir.dt.bfloat16
    i16 = mybir.dt.int16
    Alu = mybir.AluOpType
    Act = mybir.ActivationFunctionType
    AX = mybir.AxisListType

    NSLOT, NG, Cpad = plan.NSLOT, plan.NG, plan.Cpad
    GS = CORES * NSLOT
    PACKS = GS // 4
    CWk = min(CW, NSLOT)  # frontend chunk width (mini graphs are smaller)
    NCH = NSLOT // CWk

    nc = bacc.Bacc("TRN2", target_bir_lowering=False, debug=False,
                   num_devices=CORES, num_swdge_queues=4)

    def din(name, shape, dt=f32):
        return nc.dram_tensor(name, shape, dt, kind="ExternalInput")

    xin = din("xin", [32, NSLOT], bf16)
    cin = din("cin", [5, NSLOT], bf16)
    invrow = din("invrow", [1, NSLOT], bf16)    # rsqrt(deg) per slot
    invrowf = din("invrowf", [1, NSLOT])        # f32 copy for fe3
    invpg_d = din("invpg", [128, NG, 1])        # rsqrt(deg) per (p, g)
    idxw_d = din("idxw", [128, Cpad * 8], i16)
    mask_d = din("maskq", [128, Cpad, 4, 1], bf16)
    valid_d = din("valid", [128, Cpad, 1], bf16)
    wnames = {
        "cg_w1": [5, 32], "cg_b1": [32, 1], "cg_w2": [32, 32], "cg_b2": [32, 1],
        "cg_aw": [32, 32], "cg_ab": [32, 1], "base_imp": [32, 1],
        "ln_g": [32, 1], "ln_b": [32, 1],
        "fe_w1": [32, 64], "fe_b1": [64, 1], "fe_w2": [64, 64], "fe_b2": [64, 1],
        "gcn1_w": [64, 64], "gat_w": [64, 64], "asrc_st": [64, 2],
        "adst_st": [64, 2], "gcn2_w": [64, 32], "sp_w1": [32, 16],
        "sp_w2": [16, 1],
        "sp_b1": [16, 1], "sp_b2": [1, 1],
        "gcn1_b_r": [128, 1, 64], "gat_b_r": [128, 1, 64],
        "gcn2_b_r": [128, 1, 32],
    }
    for i, F in ((1, 64), (2, 64), (3, 32)):
        for nm in "gbmv":
            wnames[f"bn{i}_{nm}_r"] = [128, 1, F]
    wd = {k: din(k, s) for k, s in wnames.items()}
    w16names = {"cg_w1": [5, 32], "cg_w2": [32, 32], "cg_aw": [32, 32],
                "fe_w1": [32, 64], "fe_w2": [64, 64], "gcn1_w": [64, 64]}
    wd16 = {k: din(k + "_h", sh, bf16) for k, sh in w16names.items()}

    svi_out = nc.dram_tensor("svi", [NG, 128], f32, kind="ExternalOutput")
    dbg = os.environ.get("KERNEL_DEBUG", "0") == "1"
    if dbg:
        dbg_x1 = nc.dram_tensor("dbg_x1", [128, NG, 64], bf16, kind="ExternalOutput")
        dbg_x2 = nc.dram_tensor("dbg_x2", [128, NG, 64], bf16, kind="ExternalOutput")
        dbg_x3 = nc.dram_tensor("dbg_x3", [128, NG, 32], bf16, kind="ExternalOutput")

    cc1_in = nc.dram_tensor("cc1_in", [NSLOT // 4, 128], f32, kind="Internal")
    table1 = nc.dram_tensor("table1", [PACKS, 128], f32, kind="Internal",
                            addr_space="Shared")
    cc2_in = nc.dram_tensor("cc2_in", [NSLOT // 4, 192], f32, kind="Internal")
    table2 = nc.dram_tensor("table2", [PACKS, 192], f32, kind="Internal",
                            addr_space="Shared")
    cc3_in = nc.dram_tensor("cc3_in", [NSLOT // 4, 64], f32, kind="Internal")
    table3 = nc.dram_tensor("table3", [PACKS, 64], f32, kind="Internal",
                            addr_space="Shared")

    RG = [list(range(CORES))]

    with tile.TileContext(nc) as tc:
        with (
            tc.tile_pool(name="resident", bufs=1) as rp,
            tc.tile_pool(name="work", bufs=2) as wp,
            tc.tile_pool(name="convwork", bufs=3) as wpc,
            tc.tile_pool(name="gatherwork", bufs=2) as wpg,
            tc.tile_pool(name="psum", bufs=2, space="PSUM") as pp,
            tc.tile_pool(name="psumT", bufs=1, space="PSUM") as ppt,
        ):
            consts = rp.tile([128, 1920], f32, tag="consts", name="consts")
            _cur = [0]

            def calloc(P, W):
                c0 = _cur[0]
                _cur[0] += W
                assert _cur[0] <= 1920
                return consts[0:P, c0:c0 + W]

            def cload(name):
                sh = wd[name].shape
                P = sh[0]
                Wn = int(np.prod(sh[1:]))
                sl = calloc(P, Wn)
                nc.sync.dma_start(sl, wd[name].ap().rearrange(
                    {2: "a b -> a b", 3: "a b c -> a (b c)",
                     4: "a b c d -> a (b c d)"}[len(sh)]))
                view = sl
                if len(sh) == 3:
                    view = sl.rearrange("a (b c) -> a b c", b=sh[1])
                elif len(sh) == 4:
                    view = sl.rearrange("a (b c d) -> a b c d", b=sh[1], c=sh[2])
                return view

            W = {k: cload(k) for k in wnames}

            consts16 = rp.tile([128, 384], bf16, tag="consts16",
                               name="consts16")
            _cur16 = [0]

            def cload16(name):
                sh = wd16[name].shape
                Wn = int(np.prod(sh[1:]))
                c0 = _cur16[0]
                _cur16[0] += Wn
                assert _cur16[0] <= 288
                sl = consts16[0:sh[0], c0:c0 + Wn]
                nc.sync.dma_start(sl, wd16[name].ap())
                return sl

            W16 = {k: cload16(k) for k in w16names}
            ones16 = consts16[0:128, 288:289]
            nc.vector.memset(ones16, 1.0)
            onesr16 = consts16[0:1, 289:353]
            nc.vector.memset(onesr16, 1.0)

            ident = calloc(128, 128)
            make_identity(nc, ident)
            ones_col = calloc(128, 1)
            nc.vector.memset(ones_col, 1.0)
            ones_row64 = calloc(1, 64)
            nc.vector.memset(ones_row64, 1.0)

            def bn_scale_shift(i, F, bias_r):
                """Returns sc, sh2 with gcn/gat bias folded into the shift."""
                sc = calloc(128, F).rearrange("a (b c) -> a b c", b=1)
                sh = calloc(128, F).rearrange("a (b c) -> a b c", b=1)
                t = calloc(128, F).rearrange("a (b c) -> a b c", b=1)
                nc.vector.tensor_scalar(t, W[f"bn{i}_v_r"], EPS, None, Alu.add)
                nc.vector.reciprocal(t, t)
                nc.scalar.activation(t, t, Act.Sqrt)
                nc.vector.tensor_tensor(out=sc, in0=W[f"bn{i}_g_r"], in1=t,
                                        op=Alu.mult)
                # sh2 = b - m*sc + bias*sc = b + (bias - m)*sc
                nc.vector.tensor_tensor(out=t, in0=bias_r, in1=W[f"bn{i}_m_r"],
                                        op=Alu.subtract)
                nc.vector.tensor_tensor(out=t, in0=t, in1=sc, op=Alu.mult)
                nc.vector.tensor_tensor(out=sh, in0=W[f"bn{i}_b_r"], in1=t,
                                        op=Alu.add)
                return sc, sh

            inv_pg = rp.tile([128, NG, 1], f32, tag="invpg", name="invpg")
            nc.sync.dma_start(inv_pg[:], invpg_d.ap())

            s_mask = rp.tile([128, Cpad, 4, 1], bf16, tag="mask", name="mask")
            nc.sync.dma_start(s_mask[:], mask_d.ap())
            s_valid = rp.tile([128, Cpad, 1], bf16, tag="valid", name="valid")
            nc.sync.dma_start(s_valid[:], valid_d.ap())

            def mm(lhsT_ap, rhs_ap, m, w, tag="mmps", fast=False):
                ps = pp.tile([m, w], f32, tag=tag, name=tag)
                nc.tensor.matmul(ps[:], lhsT_ap, rhs_ap, start=True,
                                 stop=True)
                return ps

            def bcast_row(row_ap, F, w, fast=False):
                return mm(ones_row64[:, :F], row_ap, F, w, tag="bcps")

            def bcast_row16(row_ap, F, w):
                return mm(onesr16[:, :F], row_ap, F, w, tag="bcps")

            def psum_sum32(rhs_ap, w, fast=False):
                return mm(ones16[0:32, :], rhs_ap, 1, w, tag="s32ps")

            def transpose_to(in_ap, k, m, tag="tpps"):
                ps = ppt.tile([m, k], f32, tag=tag, name=tag)
                nc.tensor.transpose(ps[:], in_ap, ident[0:k, 0:k])
                return ps

            # group g's acc is complete once the window covering column
            # offs[g+1]-1 has been accumulated; emit per-group epilogues
            # (bn+relu, next table build, head) right there so vector/PE
            # work overlaps the gpsimd-bound gather descgen of later windows
            done_after = {}
            for g in range(NG):
                wi = (int(plan.offs[g + 1]) - 1) // WIN
                done_after.setdefault(wi, []).append(g)

            # frontend scratch: 4 overlaid data slots (lifetimes disjoint)
            # plus 7 single-row stat slots, all 256 wide at base partition 0
            FOFF = dict(A=0, B=256, C=512, D=768,
                        st0=1024, st1=1280, st2=1536, st3=1792, st4=2048,
                        st5=2304, st6=2560)

            def scf_tile():
                return wp.tile([128, 2816], bf16, tag="scfF", name="scfF")

            def scfh_tile():
                return wp.tile([128, 256], f32, tag="scfH", name="scfH")

            # per-group scratch (128-wide slots)
            GOFF = dict(xt=0, stk=128, h2=256, adc=384, st=512, nm=640)

            def scg_tile():
                return wp.tile([128, 768], f32, tag="scfG", name="scfG")

            # ---------------- frontend + table1 ----------------
            x1 = rp.tile([128, NG, 64], f32, tag="bigB", name="x1")
            h1_own = rp.tile([128, NG, 64], bf16, tag="ownA", name="h1_own")

            lp = nc.allow_low_precision(
                reason="bf16 frontend; rel tolerance is 2e-2")
            lp.__enter__()
            for ci in range(NCH):
                c0 = ci * CWk
                w = CWk
                scf = scf_tile()

                def S(nm, P, ww=None):
                    return scf[0:P, FOFF[nm]:FOFF[nm] + (ww or w)]

                def ST(i):
                    return scf[0:1, FOFF[f"st{i}"]:FOFF[f"st{i}"] + w]

                xc = S("A", 32)
                nc.sync.dma_start(xc, xin.ap()[:, c0:c0 + w])
                ctc = S("B", 5)
                nc.sync.dma_start(ctc, cin.ap()[:, c0:c0 + w])
                invc = ST(6)
                nc.sync.dma_start(invc, invrow.ap()[:, c0:c0 + w])

                ps = mm(W16["cg_w1"], ctc, 32, w)
                ce1 = S("C", 32)
                nc.scalar.activation(ce1, ps[:], Act.Relu, bias=W["cg_b1"])
                ps = mm(W16["cg_w2"], ce1, 32, w)
                ce2 = S("D", 32)
                nc.vector.tensor_scalar(ce2, ps[:], W["cg_b2"], None, Alu.add)
                ps = mm(W16["cg_aw"], ce2, 32, w)
                ez = S("C", 32)  # ce1 dead
                nc.scalar.activation(ez, ps[:], Act.Exp, bias=W["cg_ab"])
                psS = psum_sum32(ez, w)
                rS = ST(0)
                nc.vector.reciprocal(rS, psS[:])
                psb = bcast_row16(rS, 32, w)
                att = S("B", 32)  # ctc dead
                nc.vector.tensor_tensor(out=att, in0=ez, in1=psb[:], op=Alu.mult)
                xg = S("C", 32)  # ez dead
                nc.vector.scalar_tensor_tensor(out=xg, in0=att,
                                               scalar=W["base_imp"], in1=xc,
                                               op0=Alu.mult, op1=Alu.mult)
                ps1 = psum_sum32(xg, w)
                sq = S("B", 32)  # att dead
                nc.vector.tensor_tensor(out=sq, in0=xg, in1=xg, op=Alu.mult)
                ps2 = psum_sum32(sq, w)
                mu = ST(1)
                nc.vector.tensor_scalar(mu, ps1[:], 1.0 / 32, None, Alu.mult)
                var = ST(2)
                nc.vector.tensor_scalar(var, ps2[:], 1.0 / 32, None, Alu.mult)
                musq = ST(3)
                nc.vector.tensor_tensor(out=musq, in0=mu, in1=mu, op=Alu.mult)
                nc.vector.tensor_tensor(out=var, in0=var, in1=musq,
                                        op=Alu.subtract)
                nc.vector.tensor_scalar(var, var, EPS, None, Alu.add)
                nc.vector.reciprocal(var, var)
                rstd = ST(4)
                nc.scalar.activation(rstd, var, Act.Sqrt)
                mrs = ST(5)
                nc.vector.tensor_tensor(out=mrs, in0=mu, in1=rstd, op=Alu.mult)
                psA = bcast_row16(rstd, 32, w)
                xn = S("A", 32)  # xc dead
                nc.vector.tensor_tensor(out=xn, in0=xg, in1=psA[:], op=Alu.mult)
                psB = bcast_row16(mrs, 32, w)
                nc.vector.tensor_tensor(out=xn, in0=xn, in1=psB[:],
                                        op=Alu.subtract)
                nc.vector.tensor_scalar(xn, xn, W["ln_g"], W["ln_b"],
                                        Alu.mult, Alu.add)
                ps = mm(W16["fe_w1"], xn, 64, w)
                f1 = S("B", 64)  # sq dead
                nc.scalar.activation(f1, ps[:], Act.Relu, bias=W["fe_b1"])
                ps = mm(W16["fe_w2"], f1, 64, w)
                x0 = S("C", 64)  # xg dead
                nc.scalar.activation(x0, ps[:], Act.Relu, bias=W["fe_b2"])
                ps = mm(W16["gcn1_w"], x0, 64, w)
                psI = bcast_row16(invc, 64, w)
                scfh = scfh_tile()
                h1 = scfh[0:64, 0:w]
                nc.vector.tensor_copy(out=h1, in_=ps[:])
                nc.vector.tensor_tensor(out=h1, in0=h1, in1=psI[:],
                                        op=Alu.mult)
                # keep own (pre-scaled) h1 rows for the self-loop term
                for half in range(CWk // 128):
                    tp = transpose_to(h1[:, half * 128:(half + 1) * 128],
                                      64, 128)
                    nc.vector.tensor_copy(
                        out=h1_own[:, (CWk // 128) * ci + half], in_=tp[:])
                # table1 chunk: 4-node bf16 pack rows
                pk = wp.tile([CWk // 4, 256], bf16, tag="pk1", name="pk1")
                for q in range(4):
                    tp = transpose_to(h1[:, q::4], 64, CWk // 4)
                    nc.vector.tensor_copy(out=pk[:, q * 64:q * 64 + 64],
                                          in_=tp[:])
                nc.sync.dma_start(
                    cc1_in.ap().bitcast(bf16)[c0 // 4:c0 // 4 + CWk // 4, :],
                    pk[:])

            lp.__exit__(None, None, None)
            nc.gpsimd.collective_compute(
                "AllGather", Alu.bypass, replica_groups=RG,
                ins=[cc1_in.ap()], outs=[table1.ap()])

            # ---------------- generic GCN conv ----------------
            def gcn_conv(table, F, acc, after_group=None):
                nc.vector.memset(acc, 0.0)
                for wi, ov in enumerate(plan.wins):
                    c0 = wi * WIN
                    idxt = wpg.tile([128, WIN * 8], i16, tag="idxt", name="idxt")
                    nc.sync.dma_start(idxt[:],
                                      idxw_d.ap()[:, c0 * 8:(c0 + WIN) * 8])
                    gt = wpg.tile([128, WIN * 2 * F], f32, tag="gt", name="gt")
                    nc.gpsimd.dma_gather(
                        gt[:].rearrange("p (w e) -> p w e", e=2 * F),
                        table.ap(), idxt[:], 128 * WIN, 128 * WIN, 2 * F,
                        queue_num=wi % 4)
                    mq = wpc.tile([128, WIN, 4, F], bf16, tag="mq", name="mq")
                    nc.vector.tensor_tensor(
                        out=mq[:],
                        in0=gt[:].bitcast(bf16).rearrange(
                            "p (w q f) -> p w q f", w=WIN, q=4),
                        in1=s_mask[:, c0:c0 + WIN].to_broadcast(
                            [128, WIN, 4, F]),
                        op=Alu.mult)
                    scw = wpc.tile([128, 512], f32, tag="scw", name="scw")
                    for oi, (g, a, b) in enumerate(ov):
                        red = scw[:, 322 + 64 * (oi % 2):322 + 64 * (oi % 2) + F]
                        nc.vector.tensor_reduce(
                            out=red,
                            in_=mq[:, a:b].rearrange("p c q f -> p f c q"),
                            axis=AX.XY, op=Alu.add)
                        nc.vector.tensor_tensor(out=acc[:, g], in0=acc[:, g],
                                                in1=red, op=Alu.add)
                    if after_group is not None:
                        for g in done_after.get(wi, []):
                            after_group(g)

            def post_gcn_group(acc, x, g, F, h_own, sc, sh2):
                scw = wp.tile([128, 512], f32, tag="scwp", name="scwp")
                t = scw[:, 0:F].rearrange("p (c f) -> p c f", c=1)
                # self loop: acc + own prescaled h, then * rsqrt(deg_dst)
                nc.vector.tensor_tensor(
                    out=t, in0=acc[:, g:g + 1, 0:F],
                    in1=h_own[:, g:g + 1, 0:F], op=Alu.add)
                xg = x[:, g:g + 1, 0:F]
                nc.vector.scalar_tensor_tensor(
                    out=t, in0=t, scalar=inv_pg[:, g],
                    in1=sc.to_broadcast([128, 1, F]),
                    op0=Alu.mult, op1=Alu.mult)
                nc.vector.tensor_tensor(
                    out=xg, in0=t, in1=sh2.to_broadcast([128, 1, F]),
                    op=Alu.add)
                nc.scalar.activation(xg, xg, Act.Relu)

            # ---------------- conv1 + interleaved table2 build ----------
            ad_pg = rp.tile([128, NG, 2], f32, tag="adpg", name="adpg")
            sc1, sh1 = bn_scale_shift(1, 64, W["gcn1_b_r"])
            h2_own = rp.tile([128, NG, 66], bf16, tag="ownB", name="h2_own")

            def fe2_chunk(g):
                scb = scg_tile()
                xt = scb[0:64, GOFF["xt"]:GOFF["xt"] + 128]
                tp = transpose_to(x1[:, g], 128, 64)
                nc.vector.tensor_copy(out=xt, in_=tp[:])
                psh = mm(W["gat_w"], xt, 64, 128)
                stk = scb[0:66, GOFF["stk"]:GOFF["stk"] + 128]
                nc.vector.tensor_copy(out=stk[0:64], in_=psh[:])
                psa = mm(W["asrc_st"], stk[0:64], 2, 128)
                nc.vector.tensor_copy(out=stk[64:66], in_=psa[:])
                psd = mm(W["adst_st"], stk[0:64], 2, 128)
                adc = scb[0:2, GOFF["adc"]:GOFF["adc"] + 128]
                nc.vector.tensor_copy(out=adc, in_=psd[:])
                tpd = transpose_to(adc, 2, 128)
                nc.vector.tensor_copy(out=ad_pg[:, g], in_=tpd[:])
                tph = transpose_to(stk, 66, 128)
                nc.vector.tensor_copy(out=h2_own[:, g], in_=tph[:])
                # table2 chunk: 4-node bf16 pack rows, 96 cols per node
                pk = wp.tile([32, 384], bf16, tag="pk2", name="pk2")
                nc.vector.memset(pk[:], 0.0)
                for q in range(4):
                    tp2 = transpose_to(stk[:, q::4], 66, 32)
                    nc.vector.tensor_copy(out=pk[:, q * 96:q * 96 + 66],
                                          in_=tp2[:])
                nc.sync.dma_start(cc2_in.ap().bitcast(bf16)[g * 32:(g + 1) * 32, :],
                                  pk[:])

            acc1 = rp.tile([128, NG, 64], f32, tag="bigA", name="acc1")

            def post1_and_fe2(g):
                post_gcn_group(acc1, x1, g, 64, h1_own, sc1, sh1)
                fe2_chunk(g)

            gcn_conv(table1, 64, acc1[:], after_group=post1_and_fe2)
            if dbg:
                nc.sync.dma_start(dbg_x1.ap(), x1[:])

            # expand ad_pg to per-column layout (overlaps AllGather)
            ad_col = rp.tile([128, Cpad, 2], bf16, tag="adcol", name="adcol")
            if plan.C < Cpad:
                nc.vector.memset(ad_col[:, plan.C:Cpad], 0.0)
            for g in range(NG):
                a, b = int(plan.offs[g]), int(plan.offs[g + 1])
                nc.vector.tensor_copy(
                    out=ad_col[:, a:b],
                    in_=ad_pg[:, g:g + 1].to_broadcast([128, b - a, 2]))

            nc.gpsimd.collective_compute(
                "AllGather", Alu.bypass, replica_groups=RG,
                ins=[cc2_in.ap()], outs=[table2.ap()])

            # ---------------- GAT conv + interleaved table3 build --------
            N_pg = rp.tile([128, NG, 64], f32, tag="bigA", name="N_pg")
            nc.vector.memset(N_pg[:], 0.0)
            S_pg = rp.tile([128, NG, 2], f32, tag="spg", name="S_pg")
            nc.vector.memset(S_pg[:], 0.0)
            sc2, sh2g = bn_scale_shift(2, 64, W["gat_b_r"])
            rS2 = rp.tile([128, NG, 2], f32, tag="rs2", name="rS2")
            x2 = rp.tile([128, NG, 64], f32, tag="bigB", name="x2")
            h3_own = rp.tile([128, NG, 64], bf16, tag="ownA", name="h3_own")

            def fe3_chunk(g):
                scb = scg_tile()
                scg = scb
                xt = scb[0:64, GOFF["xt"]:GOFF["xt"] + 128]
                tp = transpose_to(x2[:, g], 128, 64)
                nc.vector.tensor_copy(out=xt, in_=tp[:])
                ps = mm(W["gcn2_w"], xt, 32, 128)
                invc = scg[0:1, GOFF["st"]:GOFF["st"] + 128]
                nc.sync.dma_start(invc, invrowf.ap()[:, g * 128:(g + 1) * 128])
                psI = bcast_row(invc, 32, 128)
                h3 = scb[0:32, GOFF["h2"]:GOFF["h2"] + 128]
                nc.vector.tensor_copy(out=h3, in_=ps[:])
                nc.vector.tensor_tensor(out=h3, in0=h3, in1=psI[:],
                                        op=Alu.mult)
                tph = transpose_to(h3, 32, 128)
                nc.vector.tensor_copy(out=h3_own[:, g, 0:32], in_=tph[:])
                pk = wp.tile([32, 128], bf16, tag="pk3", name="pk3")
                for q in range(4):
                    tp2 = transpose_to(h3[:, q::4], 32, 32)
                    nc.vector.tensor_copy(out=pk[:, q * 32:q * 32 + 32],
                                          in_=tp2[:])
                nc.sync.dma_start(cc3_in.ap().bitcast(bf16)[g * 32:(g + 1) * 32, :],
                                  pk[:])

            def post2_and_fe3(g):
                scw = wp.tile([128, 512], f32, tag="scwp", name="scwp")
                # self-loop alpha term
                es = scw[:, 0:2].rearrange("p (c h) -> p c h", c=1)
                nc.vector.tensor_tensor(
                    out=es, in0=h2_own[:, g:g + 1, 64:66],
                    in1=ad_pg[:, g:g + 1], op=Alu.add)
                es2 = scw[:, 2:4].rearrange("p (c h) -> p c h", c=1)
                nc.vector.tensor_scalar(es2, es, 0.2, None, Alu.mult)
                nc.vector.tensor_tensor(out=es, in0=es, in1=es2, op=Alu.max)
                nc.scalar.activation(es, es, Act.Exp)
                nc.vector.tensor_tensor(out=S_pg[:, g], in0=S_pg[:, g],
                                        in1=es[:, 0], op=Alu.add)
                for h in range(2):
                    nc.vector.scalar_tensor_tensor(
                        out=N_pg[:, g:g + 1, 32 * h:32 * h + 32],
                        in0=h2_own[:, g:g + 1, 32 * h:32 * h + 32],
                        scalar=es[:, 0, h:h + 1],
                        in1=N_pg[:, g:g + 1, 32 * h:32 * h + 32],
                        op0=Alu.mult, op1=Alu.add)
                rg = rS2[:, g:g + 1]
                nc.vector.tensor_scalar(rg, S_pg[:, g:g + 1], 1e-16, None,
                                        Alu.add)
                nc.vector.reciprocal(rg, rg)
                xg = x2[:, g:g + 1]
                for h in range(2):
                    nc.vector.scalar_tensor_tensor(
                        out=xg[:, :, 32 * h:32 * h + 32],
                        in0=N_pg[:, g:g + 1, 32 * h:32 * h + 32],
                        scalar=rg[:, :, h],
                        in1=sc2[:, :, 32 * h:32 * h + 32].to_broadcast(
                            [128, 1, 32]),
                        op0=Alu.mult, op1=Alu.mult)
                nc.vector.tensor_tensor(
                    out=xg, in0=xg, in1=sh2g.to_broadcast([128, 1, 64]),
                    op=Alu.add)
                nc.scalar.activation(xg, xg, Act.Relu)
                fe3_chunk(g)

            for wi, ov in enumerate(plan.wins):
                c0 = wi * WIN
                idxt = wpg.tile([128, WIN * 8], i16, tag="idxt", name="idxt")
                nc.sync.dma_start(idxt[:], idxw_d.ap()[:, c0 * 8:(c0 + WIN) * 8])
                gt = wpg.tile([128, WIN * 192], f32, tag="gt", name="gt")
                nc.gpsimd.dma_gather(
                    gt[:].rearrange("p (w e) -> p w e", e=192), table2.ap(),
                    idxt[:], 128 * WIN, 128 * WIN, 192, queue_num=wi % 4)
                mq = wpc.tile([128, WIN, 4, 66], bf16, tag="mq", name="mq")
                gtb = gt[:].bitcast(bf16).rearrange(
                    "p (w q f) -> p w q f", w=WIN, q=4)
                nc.vector.tensor_tensor(
                    out=mq[:], in0=gtb[:, :, :, 0:66],
                    in1=s_mask[:, c0:c0 + WIN].to_broadcast([128, WIN, 4, 66]),
                    op=Alu.mult)
                sel = wpc.tile([128, WIN, 66], f32, tag="sel", name="sel")
                nc.vector.tensor_reduce(
                    out=sel[:], in_=mq[:].rearrange("p c q f -> p c f q"),
                    axis=AX.X, op=Alu.add)
                scw = wpc.tile([128, 512], f32, tag="scw", name="scw")
                # whole-window attention math (no per-ov splitting)
                e = scw[:, 0:2 * WIN].rearrange("p (c h) -> p c h", h=2)
                nc.vector.tensor_tensor(
                    out=e, in0=sel[:, :, 64:66],
                    in1=ad_col[:, c0:c0 + WIN], op=Alu.add)
                e2 = scw[:, 2 * WIN:4 * WIN].rearrange("p (c h) -> p c h", h=2)
                nc.vector.tensor_scalar(e2, e, 0.2, None, Alu.mult)
                nc.vector.tensor_tensor(out=e, in0=e, in1=e2, op=Alu.max)
                nc.scalar.activation(e, e, Act.Exp)
                nc.vector.tensor_tensor(
                    out=e, in0=e,
                    in1=s_valid[:, c0:c0 + WIN].to_broadcast([128, WIN, 2]),
                    op=Alu.mult)
                nmt = wpc.tile([128, WIN, 2, 32], bf16, tag="nmt", name="nmt")
                e4 = scw[:, 0:2 * WIN].rearrange("p (c h f) -> p c h f",
                                                 h=2, f=1)
                nc.vector.tensor_tensor(
                    out=nmt[:],
                    in0=sel[:, :, 0:64].rearrange("p c (h f) -> p c h f", h=2),
                    in1=e4.to_broadcast([128, WIN, 2, 32]),
                    op=Alu.mult)
                red0 = 4 * WIN
                for (g, a, b) in ov:
                    red2 = scw[:, red0:red0 + 2]
                    nc.vector.tensor_reduce(
                        out=red2, in_=e[:, a:b].rearrange("p c h -> p h c"),
                        axis=AX.X, op=Alu.add)
                    nc.vector.tensor_tensor(out=S_pg[:, g], in0=S_pg[:, g],
                                            in1=red2, op=Alu.add)
                    redh = scw[:, red0 + 2:red0 + 66]
                    nc.vector.tensor_reduce(
                        out=redh.rearrange("p (h f) -> p h f", h=2),
                        in_=nmt[:, a:b].rearrange("p c h f -> p h f c"),
                        axis=AX.X, op=Alu.add)
                    nc.vector.tensor_tensor(out=N_pg[:, g], in0=N_pg[:, g],
                                            in1=redh, op=Alu.add)
                for g in done_after.get(wi, []):
                    post2_and_fe3(g)

            if dbg:
                nc.sync.dma_start(dbg_x2.ap(), x2[:])

            nc.gpsimd.collective_compute(
                "AllGather", Alu.bypass, replica_groups=RG,
                ins=[cc3_in.ap()], outs=[table3.ap()])

            # ---------------- conv3 + interleaved head ----------------
            sc3, sh3 = bn_scale_shift(3, 32, W["gcn2_b_r"])
            x3 = rp.tile([128, NG, 64], f32, tag="bigB", name="x3")
            acc3 = rp.tile([128, NG, 64], f32, tag="bigA", name="acc3")

            def head_chunk(g):
                scb = scg_tile()
                scg = scb
                tp = transpose_to(x3[:, g, 0:32], 128, 32)
                xh = scb[0:32, GOFF["xt"]:GOFF["xt"] + 128]
                nc.vector.tensor_copy(out=xh, in_=tp[:])
                ps = mm(W["sp_w1"], xh, 16, 128)
                hh = scb[0:16, GOFF["stk"]:GOFF["stk"] + 128]
                nc.scalar.activation(hh, ps[:], Act.Relu, bias=W["sp_b1"])
                ps = mm(W["sp_w2"], hh, 1, 128)
                sv = scg[0:1, GOFF["adc"]:GOFF["adc"] + 128]
                nc.scalar.activation(sv, ps[:], Act.Sigmoid, bias=W["sp_b2"])
                nc.sync.dma_start(svi_out.ap()[g:g + 1, :], sv)

            def post3_and_head(g):
                post_gcn_group(acc3, x3, g, 32, h3_own, sc3, sh3)
                head_chunk(g)

            gcn_conv(table3, 32, acc3[:, :, 0:32], after_group=post3_and_head)
            if dbg:
                nc.sync.dma_start(dbg_x3.ap(), x3[:, :, 0:32])

    nc.compile()
    return nc


def _make_inputs(plan, inputs):
    import ml_dtypes
    bf16 = ml_dtypes.bfloat16

    NSLOT = plan.NSLOT
    xf = inputs["accessibility_features"].astype(np.float32)
    cf = inputs["context_features"].astype(np.float32)

    def col(a):
        return np.ascontiguousarray(np.asarray(a, np.float32).reshape(-1, 1))

    def rep(a, shape):
        return np.ascontiguousarray(
            np.broadcast_to(np.asarray(a, np.float32), shape))

    common = {
        "cg_w1": np.asarray(inputs["cg_w1"], np.float32),
        "cg_b1": col(inputs["cg_b1"]),
        "cg_w2": np.asarray(inputs["cg_w2"], np.float32),
        "cg_b2": col(inputs["cg_b2"]),
        "cg_aw": np.asarray(inputs["cg_aw"], np.float32),
        "cg_ab": col(inputs["cg_ab"]),
        "base_imp": col(inputs["base_imp"]),
        "ln_g": col(inputs["ln_g"]), "ln_b": col(inputs["ln_b"]),
        "fe_w1": np.asarray(inputs["fe_w1"], np.float32),
        "fe_b1": col(inputs["fe_b1"]),
        "fe_w2": np.asarray(inputs["fe_w2"], np.float32),
        "fe_b2": col(inputs["fe_b2"]),
        "gcn1_w": np.asarray(inputs["gcn1_w"], np.float32),
        "cg_w1_h": np.asarray(inputs["cg_w1"], bf16),
        "cg_w2_h": np.asarray(inputs["cg_w2"], bf16),
        "cg_aw_h": np.asarray(inputs["cg_aw"], bf16),
        "fe_w1_h": np.asarray(inputs["fe_w1"], bf16),
        "fe_w2_h": np.asarray(inputs["fe_w2"], bf16),
        "gcn1_w_h": np.asarray(inputs["gcn1_w"], bf16),
        "gat_w": np.asarray(inputs["gat_w"], np.float32),
        "gcn2_w": np.asarray(inputs["gcn2_w"], np.float32),
        "sp_w1": np.asarray(inputs["sp_w1"], np.float32),
        "sp_b1": col(inputs["sp_b1"]),
        "sp_w2": np.asarray(inputs["sp_w2"], np.float32),
        "sp_b2": col(inputs["sp_b2"]),
        "gcn1_b_r": rep(inputs["gcn1_b"][None, None, :], (128, 1, 64)),
        "gat_b_r": rep(inputs["gat_b"][None, None, :], (128, 1, 64)),
        "gcn2_b_r": rep(inputs["gcn2_b"][None, None, :], (128, 1, 32)),
    }
    asrc = np.asarray(inputs["gat_asrc"], np.float32)
    ast = np.zeros((64, 2), np.float32)
    ast[0:32, 0] = asrc[0]
    ast[32:64, 1] = asrc[1]
    common["asrc_st"] = ast
    adst = np.asarray(inputs["gat_adst"], np.float32)
    adt2 = np.zeros((64, 2), np.float32)
    adt2[0:32, 0] = adst[0]
    adt2[32:64, 1] = adst[1]
    common["adst_st"] = adt2
    for i in (1, 2, 3):
        F = 32 if i == 3 else 64
        for nm in "gbmv":
            common[f"bn{i}_{nm}_r"] = rep(
                np.asarray(inputs[f"bn{i}_{nm}"])[None, None, :], (128, 1, F))

    maps = []
    for c in range(CORES):
        o = plan.orders[c]
        m = o >= 0
        xs = np.zeros((NSLOT, 32), np.float32)
        cs = np.zeros((NSLOT, 5), np.float32)
        xs[m] = xf[o[m]]
        cs[m] = cf[o[m]]
        im = dict(common)
        im["xin"] = np.ascontiguousarray(xs.T).astype(bf16)
        im["cin"] = np.ascontiguousarray(cs.T).astype(bf16)
        im["invrow"] = plan.inv_row[c].astype(bf16)
        im["invrowf"] = plan.inv_row[c]
        im["invpg"] = plan.inv_pg[c]
        im["idxw"] = plan.idxw[c]
        im["maskq"] = plan.maskq[c].astype(bf16)
        im["valid"] = plan.valid[c].astype(bf16)
        maps.append(im)
    return maps


def kernel(**inputs):
    from concourse.bass_utils import run_bass_kernel_spmd

    edge_index = np.asarray(inputs["edge_index"])
    N = inputs["accessibility_features"].shape[0]
    plan = build_plan(edge_index, N)
    nc = build_kernel(plan)
    in_maps = _make_inputs(plan, inputs)

    trace = os.environ.get("KERNEL_TRACE", "0") == "1"
    res = run_bass_kernel_spmd(nc, in_maps, core_ids=list(range(CORES)),
                               trace=trace)
    kernel.last_result = res

    svi = np.zeros(N, np.float32)
    for c in range(CORES):
        o = plan.orders[c]
        m = o >= 0
        flat = res.results[c]["svi"].reshape(plan.NG * 128)
        svi[o[m]] = flat[np.nonzero(m)[0]]
    return svi
